# revision 48
# baseline (speedup 1.0000x reference)
"""GATConv x2 + LayerNorm (GNN message passing) on 8 TRN2 NeuronCores.

Strategy (edge-parallel, dst-sharded):
  - Nodes are sharded across 8 cores by id range; each core owns all edges
    whose dst falls in its range (plus self loops).
  - Host assigns each core's nodes to 128-slot "windows", balancing total
    in-degree per window; edges are laid out per window in 128-edge blocks,
    split into low/high halves by source table row (int16 gather indices).
  - Phase 0 (per core): h1 = x @ W1 for own nodes, attention dot products
    e_src1/e_dst1; packed node table rows [h1 | e_src1] -> AllGather.
  - Phase 1: per window, dma_gather source rows (round-robin over 4 SWDGE
    queues so descriptor generation overlaps across Q7 core pairs), load
    host-precomputed one-hot matrices S (edge->dstslot) and S_T
    (dstslot->edge), broadcast per-window e_dst to edges via K tiny
    matmuls with S_T stationary, extract embedded e_src with the scalar
    engine, compute exp(leaky_relu(e_src+e_dst)) in bf16, one matmul per
    128-edge block with S stationary accumulates the weighted message sum
    and softmax denominators in PSUM.  Window epilogue normalizes, applies
    relu, computes h2 = relu(out1) @ W2 and the layer-2 attention dots;
    packed table 2 -> AllGather.
  - Phase 2: same edge machinery on table 2; epilogue does head-mean,
    bias, LayerNorm, and writes the output rows (window-slot order; host
    unpermutes).

Tables store h in bf16 with e-values embedded as f32 (bitcast slices).
"""

import sys

sys.path.insert(0, "/opt/trn_rl_repo")

import math
import os
import numpy as np
import ml_dtypes

import concourse.bass as bass
import concourse.bacc as bacc
import concourse.mybir as mybir
from concourse import tile
from concourse.bass_utils import run_bass_kernel_spmd

F32 = mybir.dt.float32
BF16 = mybir.dt.bfloat16
I16 = mybir.dt.int16
I32 = mybir.dt.int32
AF = mybir.ActivationFunctionType
ALU = mybir.AluOpType

CORES = 8
WIN = 128
H = 4

NEG_SLOPE = 0.2
EPS_LN = 1e-5


class Cfg:
    def __init__(self, N, IN_DIM, C1, C2, KL, KH, CH):
        assert N % CORES == 0
        self.N = N
        self.IN = IN_DIM          # input feature dim (=128)
        self.C1 = C1              # per-head dim layer 1 (64)
        self.C2 = C2              # per-head dim layer 2 (128)
        self.F1 = H * C1          # 256
        self.F2 = H * C2          # 512
        self.NODES_PC = N // CORES
        self.NW = math.ceil(self.NODES_PC / WIN)
        self.SLOTS_PC = self.NW * WIN
        self.TOT = CORES * self.SLOTS_PC
        assert self.TOT % 2 == 0
        self.HALF = self.TOT // 2
        assert self.HALF <= 32767, self.HALF
        assert self.SLOTS_PC <= 32767
        self.KL = KL
        self.KH = KH
        self.K = KL + KH
        # chunked AllGather: table rows are chunk-major
        # row(core,w,o) = crb[chunk(w)] + core*cw[chunk]*WIN + (w-w0)*WIN + o
        self.CH = CH
        self.cw = [self.NW // CH + (1 if i < self.NW % CH else 0)
                   for i in range(CH)]
        self.w0s = np.concatenate([[0], np.cumsum(self.cw)])
        self.crb = np.concatenate(
            [[0], np.cumsum([CORES * c * WIN for c in self.cw])])[:-1]
        # packed table rows (bf16 elements, 256B-multiple bytes)
        self.R1 = _pad_row(self.F1 * 2 + 16)   # bf16 cols
        self.R2 = _pad_row(self.F2 * 2 + 16)
        self.key = (N, IN_DIM, C1, C2, KL, KH, CH)

    def grow(self, core, w, o):
        """Global table row for (core, window, offset) — vectorized."""
        w = np.asarray(w)
        ch = np.searchsorted(self.w0s, w, side="right") - 1
        cw = np.asarray(self.cw)[ch]
        w0 = self.w0s[ch]
        return self.crb[ch] + core * cw * WIN + (w - w0) * WIN + o


def _pad_row(nbytes):
    """Round row bytes up to a multiple of 256; return bf16 col count."""
    b = ((nbytes + 255) // 256) * 256
    return b // 2


def _wrap_idx(arr):
    """[NWxL] int -> [128, NW*L/16] int16 wrapped+replicated per call."""
    nw, L = arr.shape
    assert L % 16 == 0
    w = arr.reshape(nw, L // 16, 16).transpose(0, 2, 1)  # [nw, 16, L/16]
    w = np.concatenate([w] * 8, axis=1)                  # [nw, 128, L/16]
    w = np.concatenate(list(w), axis=1)                  # [128, nw*L/16]
    return np.ascontiguousarray(w.astype(np.int16))


def prep(x, edge_index, W1, a_src1, a_dst1, b1, W2, a_src2, a_dst2, b2,
         gamma, beta):
    """Host-side sharding. Returns (cfg, in_maps, slot_global)."""
    N, IN_DIM = x.shape
    C1 = a_src1.shape[1]
    C2 = a_src2.shape[1]

    src = np.asarray(edge_index[0], dtype=np.int64)
    dst = np.asarray(edge_index[1], dtype=np.int64)
    loop = np.arange(N, dtype=np.int64)
    src = np.concatenate([src, loop])
    dst = np.concatenate([dst, loop])

    NODES_PC = N // CORES
    NW = math.ceil(NODES_PC / WIN)
    SLOTS_PC = NW * WIN

    # ---- window assignment per core (balance in-degree across NW bins) ----
    deg = np.bincount(dst, minlength=N)
    slot_global = np.empty(N, dtype=np.int64)
    win_of = np.empty(N, dtype=np.int64)    # window within core
    off_of = np.empty(N, dtype=np.int64)    # slot within window
    for c in range(CORES):
        nodes = np.arange(c * NODES_PC, (c + 1) * NODES_PC)
        d = deg[nodes]
        order = np.argsort(-d, kind="stable")
        # greedy: place next-heaviest node into least-loaded non-full bin
        bin_load = np.zeros(NW, dtype=np.int64)
        bin_cnt = np.zeros(NW, dtype=np.int64)
        wsel = np.empty(len(nodes), dtype=np.int64)
        osel = np.empty(len(nodes), dtype=np.int64)
        import heapq
        heap = [(0, 0, w) for w in range(NW)]
        heapq.heapify(heap)
        for i in order:
            while True:
                load, cnt, w = heapq.heappop(heap)
                if cnt < WIN:
                    break
            wsel[i] = w
            osel[i] = cnt
            heapq.heappush(heap, (load + d[i], cnt + 1, w))
        win_of[nodes] = wsel
        off_of[nodes] = osel
        slot_global[nodes] = c * SLOTS_PC + wsel * WIN + osel

    HALF = CORES * SLOTS_PC // 2
    CH = int(os.environ.get("GAT_CHUNKS", "4"))
    cfg0 = Cfg(N, IN_DIM, C1, C2, 0, 0, CH)

    owner = dst // NODES_PC
    src_core = src // NODES_PC

    # ---- pass 2: rebalance bins on (lo, hi) in-degree jointly ----
    # lo/hi labels from the pass-1 placement (approximation; final KL/KH
    # are recomputed from the final placement below)
    lo_lbl = (off_of[src] % 2) == 0
    dlo = np.bincount(dst[lo_lbl], minlength=N)
    dhi = np.bincount(dst[~lo_lbl], minlength=N)
    for c in range(CORES):
        nodes = np.arange(c * NODES_PC, (c + 1) * NODES_PC)
        dl, dh = dlo[nodes], dhi[nodes]
        order = np.argsort(-(dl + dh), kind="stable")
        binlo = np.zeros(NW, dtype=np.int64)
        binhi = np.zeros(NW, dtype=np.int64)
        bincnt = np.zeros(NW, dtype=np.int64)
        for i in order:
            cost = np.maximum(binlo + dl[i], binhi + dh[i]).astype(np.float64)
            cost[bincnt >= WIN] = np.inf
            w = int(np.argmin(cost))
            win_of[nodes[i]] = w
            off_of[nodes[i]] = bincnt[w]
            binlo[w] += dl[i]
            binhi[w] += dh[i]
            bincnt[w] += 1
        slot_global[nodes] = (c * SLOTS_PC + win_of[nodes] * WIN
                              + off_of[nodes])

    # ---- per-core edge layout (final placement) ----
    # Gathers are split by slot-offset PARITY (even/odd), which is identical
    # under both the core-major (t1) and chunk-major (t2) row numberings:
    # row = base*128 + o in both, so parity(row) = parity(o).  Each stream
    # gathers with elem_step = 2 rows, so int16 indices (row//2) cover the
    # whole table.  "lo" = even, "hi" = odd below.
    r1_of = slot_global                                  # core-major rows
    r2_of = cfg0.grow(np.arange(N) // NODES_PC, win_of, off_of)
    src_r1 = r1_of[src]
    src_r2 = r2_of[src]
    e_w = win_of[dst]
    e_off = off_of[dst]
    e_low = (src_r1 % 2) == 0

    # first pass: find KL / KH
    KL = 0
    KH = 0
    per_core = []
    for c in range(CORES):
        m = owner == c
        ew, eo, el = e_w[m], e_off[m], e_low[m]
        er1, er2 = src_r1[m] // 2, src_r2[m] // 2
        nlo = np.bincount(ew[el], minlength=NW)
        nhi = np.bincount(ew[~el], minlength=NW)
        KL = max(KL, int(np.ceil(nlo.max() / WIN)))
        KH = max(KH, int(np.ceil(nhi.max() / WIN)))
        per_core.append((ew, eo, er1, er2, el))
    cfg = Cfg(N, IN_DIM, C1, C2, KL, KH, CH)
    K = cfg.K

    in_maps = []
    for c in range(CORES):
        ew, eo, er1, er2, el = per_core[c]
        idx_lo1 = np.zeros((NW, KL * WIN), dtype=np.int64)
        idx_hi1 = np.zeros((NW, KH * WIN), dtype=np.int64)
        idx_lo2 = np.zeros((NW, KL * WIN), dtype=np.int64)
        idx_hi2 = np.zeros((NW, KH * WIN), dtype=np.int64)
        dstoff = np.full((NW, K * WIN), 999, dtype=np.int64)
        for w in range(NW):
            wm = ew == w
            lo_m = wm & el
            hi_m = wm & ~el
            # ascending source rows within each stream -> DMA reads walk
            # HBM mostly monotonically (better row-buffer locality)
            plo = np.argsort(er1[lo_m], kind="stable")
            phi = np.argsort(er1[hi_m], kind="stable")
            olo, ohi = eo[lo_m][plo], eo[hi_m][phi]
            idx_lo1[w, : len(olo)] = er1[lo_m][plo]
            idx_hi1[w, : len(ohi)] = er1[hi_m][phi]
            idx_lo2[w, : len(olo)] = er2[lo_m][plo]
            idx_hi2[w, : len(ohi)] = er2[hi_m][phi]
            dstoff[w, : len(olo)] = olo
            dstoff[w, KL * WIN : KL * WIN + len(ohi)] = ohi

        # one-hot matrices, [128, NW*K*128] bf16:
        #   Sg [e, (w,b,s)] = (dstoff[(w,b,e)] == s)   (edge-partition)
        #   SgT[s, (w,b,e)] = same                      (dstslot-partition)
        flat = dstoff.reshape(-1)                    # [(w,b,e)]
        j = np.arange(flat.size)
        valid = flat < WIN
        jv, fv = j[valid], flat[valid]
        L = NW * K * WIN
        Sg = np.zeros((WIN, L), dtype=ml_dtypes.bfloat16)
        Sg[jv % WIN, (jv // WIN) * WIN + fv] = 1
        SgT = np.zeros((WIN, L), dtype=ml_dtypes.bfloat16)
        SgT[fv, jv] = 1

        xs = np.zeros((SLOTS_PC, IN_DIM), dtype=np.float32)
        nodes = np.arange(c * NODES_PC, (c + 1) * NODES_PC)
        loc = win_of[nodes] * WIN + off_of[nodes]
        xs[loc] = np.asarray(x[nodes], dtype=np.float32)

        in_maps.append({
            "xT": np.ascontiguousarray(xs.T),
            "idx_lo1": _wrap_idx(idx_lo1),
            "idx_hi1": _wrap_idx(idx_hi1),
            "idx_lo2": _wrap_idx(idx_lo2),
            "idx_hi2": _wrap_idx(idx_hi2),
            "Sg": np.ascontiguousarray(Sg),
            "SgT": np.ascontiguousarray(SgT),
            "W1": np.asarray(W1, np.float32),
            "W2": np.asarray(W2, np.float32),
            "asrc1": np.asarray(a_src1, np.float32).reshape(1, -1),
            "adst1": np.asarray(a_dst1, np.float32).reshape(1, -1),
            "asrc2": np.asarray(a_src2, np.float32).reshape(1, -1),
            "adst2": np.asarray(a_dst2, np.float32).reshape(1, -1),
            "b1": np.asarray(b1, np.float32).reshape(1, -1),
            "b2": np.asarray(b2, np.float32).reshape(1, -1),
            "gamma": np.asarray(gamma, np.float32).reshape(1, -1),
            "beta": np.asarray(beta, np.float32).reshape(1, -1),
        })
    return cfg, in_maps, slot_global


# --------------------------------------------------------------------------
# device program
# --------------------------------------------------------------------------

def build(cfg):
    PH = os.environ.get("GAT_PHASES", "012")
    _sc = int(os.environ.get("GAT_SCRATCH", "16384"))
    _nq = int(os.environ.get("GAT_QUEUES", "4"))
    nc = bacc.Bacc("TRN2", target_bir_lowering=False, debug=False,
                   num_devices=CORES, dynamic_dma_scratch_size=_sc,
                   num_swdge_queues=_nq)
    NW, K, KL, KH = cfg.NW, cfg.K, cfg.KL, cfg.KH
    F1, F2, R1, R2 = cfg.F1, cfg.F2, cfg.R1, cfg.R2
    C1, C2 = cfg.C1, cfg.C2
    SL, TOT, HALF = cfg.SLOTS_PC, cfg.TOT, cfg.HALF
    RG = [list(range(CORES))]
    CH, cw, w0s, crb = cfg.CH, cfg.cw, cfg.w0s, cfg.crb
    chunk_end = {int(w0s[i + 1]) - 1: i for i in range(CH)}

    def emit_cc(shard, full, ch, ccc):
        w0, w1 = int(w0s[ch]), int(w0s[ch + 1])
        b = int(crb[ch])
        nc.gpsimd.collective_compute(
            "AllGather", ALU.bypass, replica_groups=RG,
            ins=[shard[w0 * WIN:w1 * WIN, :]],
            outs=[full[b:b + CORES * (w1 - w0) * WIN, :]])

    # ---- kernel I/O ----
    xT = nc.dram_tensor("xT", [cfg.IN, SL], F32, kind="ExternalInput")
    idx_lo1 = nc.dram_tensor("idx_lo1", [128, NW * KL * 8], I16, kind="ExternalInput")
    idx_hi1 = nc.dram_tensor("idx_hi1", [128, NW * KH * 8], I16, kind="ExternalInput")
    idx_lo2 = nc.dram_tensor("idx_lo2", [128, NW * KL * 8], I16, kind="ExternalInput")
    idx_hi2 = nc.dram_tensor("idx_hi2", [128, NW * KH * 8], I16, kind="ExternalInput")
    Sg = nc.dram_tensor("Sg", [128, NW * K * 128], BF16, kind="ExternalInput")
    SgT = nc.dram_tensor("SgT", [128, NW * K * 128], BF16, kind="ExternalInput")
    W1 = nc.dram_tensor("W1", [cfg.IN, F1], F32, kind="ExternalInput")
    W2 = nc.dram_tensor("W2", [F1, F2], F32, kind="ExternalInput")
    vecs = {}
    for nm, d in [("asrc1", F1), ("adst1", F1), ("asrc2", F2), ("adst2", F2),
                  ("b1", F1), ("b2", C2), ("gamma", C2), ("beta", C2)]:
        vecs[nm] = nc.dram_tensor(nm, [1, d], F32, kind="ExternalInput")
    out = nc.dram_tensor("out", [SL, C2], F32, kind="ExternalOutput")

    # ---- internal DRAM ----
    t1_shard = nc.dram_tensor("t1_shard", [SL, R1], BF16, kind="Internal")
    t2_shard = nc.dram_tensor("t2_shard", [SL, R2], BF16, kind="Internal")
    t1_full = nc.dram_tensor("t1_full", [TOT, R1], BF16, kind="Internal",
                             addr_space="Shared")
    t2_full = nc.dram_tensor("t2_full", [TOT, R2], BF16, kind="Internal",
                             addr_space="Shared")
    edst1 = nc.dram_tensor("edst1", [SL, H], BF16, kind="Internal")
    edst2 = nc.dram_tensor("edst2", [SL, H], BF16, kind="Internal")

    with tile.TileContext(nc) as tc:
        with tc.tile_pool(name="const", bufs=1) as cp:
            iota_i = cp.tile([128, 128], I32)
            nc.gpsimd.iota(iota_i[:], pattern=[[1, 128]], base=0,
                           channel_multiplier=0)
            iota_f = cp.tile([128, 128], F32)
            nc.vector.tensor_copy(iota_f[:], iota_i[:])
            ic_i = cp.tile([128, 1], I32)
            nc.gpsimd.iota(ic_i[:], pattern=[[0, 1]], base=0,
                           channel_multiplier=1)
            ic_f = cp.tile([128, 1], F32)
            nc.vector.tensor_copy(ic_f[:], ic_i[:])
            ident = cp.tile([128, 128], F32)
            nc.vector.tensor_scalar(ident[:], iota_f[:], ic_f[:, 0:1],
                                    None, ALU.is_equal)

            W1sb = cp.tile([128, F1], F32)
            nc.sync.dma_start(W1sb[:], W1[:, :])
            W2af = cp.tile([128, F2], F32)
            W2bf = cp.tile([128, F2], F32)
            nc.sync.dma_start(W2af[:], W2[0:128, :])
            nc.sync.dma_start(W2bf[:], W2[128:256, :])
            W2a = cp.tile([128, F2], BF16)
            W2b = cp.tile([128, F2], BF16)
            nc.vector.tensor_copy(W2a[:], W2af[:])
            nc.vector.tensor_copy(W2b[:], W2bf[:])
            identb = cp.tile([128, 128], BF16)
            nc.vector.tensor_copy(identb[:], ident[:])
            bcb = {}

            ones = cp.tile([1, 128], F32)
            nc.vector.memset(ones[:], 1.0)
            epsb = cp.tile([128, 1], F32)
            nc.vector.memset(epsb[:], EPS_LN)

            # broadcast small vectors to [128, D] via 1-row matmul
            bc = {}
            with tc.tile_pool(name="bcp", bufs=2, space="PSUM") as bps, \
                 tc.tile_pool(name="bcs", bufs=1) as bsb:
                for nm, d in [("asrc1", F1), ("adst1", F1), ("asrc2", F2),
                              ("adst2", F2), ("b1", F1), ("b2", C2),
                              ("gamma", C2), ("beta", C2)]:
                    vsb = bsb.tile([1, d], F32, tag="vload")
                    nc.sync.dma_start(vsb[:], vecs[nm][:, :])
                    t = cp.tile([128, d], F32, tag=f"bc_{nm}")
                    ps = bps.tile([128, d], F32, tag="bcps")
                    nc.tensor.matmul(ps[:], ones[:], vsb[:], start=True,
                                     stop=True)
                    nc.vector.tensor_copy(t[:], ps[:])
                    bc[nm] = t
                    if nm in ("asrc1", "adst1", "asrc2", "adst2"):
                        tb = cp.tile([128, d], BF16, tag=f"bcb_{nm}")
                        nc.vector.tensor_copy(tb[:], ps[:])
                        bcb[nm] = tb

            # ================= Phase 0: node tables =================
            with tc.tile_pool(name="p0", bufs=3) as p0, \
                 tc.tile_pool(name="p0ps", bufs=2, space="PSUM") as p0ps:
                for w in range(NW):
                    xt = p0.tile([128, 128], F32, tag="xt")
                    nc.sync.dma_start(xt[:], xT[:, w * WIN:(w + 1) * WIN])
                    h1 = p0ps.tile([128, F1], F32, tag="h1")
                    nc.tensor.matmul(h1[:], xt[:], W1sb[:], start=True,
                                     stop=True)
                    pk = p0.tile([128, R1], BF16, tag="pk")
                    nc.scalar.copy(pk[:, 0:F1], h1[:])
                    h1b = pk[:, 0:F1]
                    prod = p0.tile([128, F1], BF16, tag="prod")
                    es1 = p0.tile([128, H], F32, tag="es1")
                    nc.vector.tensor_tensor(prod[:], h1b, bcb["asrc1"][:],
                                            ALU.mult)
                    nc.vector.reduce_sum(
                        es1[:], prod[:].rearrange("p (h c) -> p h c", c=C1),
                        axis=mybir.AxisListType.X)
                    prod2 = p0.tile([128, F1], BF16, tag="prod2")
                    ed1f = p0.tile([128, H], F32, tag="ed1f")
                    ed1 = p0.tile([128, H], BF16, tag="ed1")
                    nc.vector.tensor_tensor(prod2[:], h1b, bcb["adst1"][:],
                                            ALU.mult)
                    nc.vector.reduce_sum(
                        ed1f[:], prod2[:].rearrange("p (h c) -> p h c", c=C1),
                        axis=mybir.AxisListType.X)
                    nc.scalar.copy(ed1[:], ed1f[:])
                    nc.vector.tensor_copy(
                        pk[:, F1:F1 + 8].bitcast(F32), es1[:])
                    nc.sync.dma_start(
                        t1_shard[w * WIN:(w + 1) * WIN, 0:F1 + 8],
                        pk[:, 0:F1 + 8])
                    nc.sync.dma_start(edst1[w * WIN:(w + 1) * WIN, :],
                                      ed1[:])
            if "1" in PH or "2" in PH:
                # t1 is core-major: one fast AllGather (plain concat)
                nc.gpsimd.collective_compute(
                    "AllGather", ALU.bypass, replica_groups=RG,
                    ins=[t1_shard[:, :]], outs=[t1_full[:, :]])

            # ================= Phase 1 and 2 =================
            qn = [0]

            def edge_phase(layer):
                F = F1 if layer == 1 else F2
                C = C1 if layer == 1 else C2
                R = R1 if layer == 1 else R2
                tfull = t1_full if layer == 1 else t2_full
                edst = edst1 if layer == 1 else edst2
                sfx = f"L{layer}"
                _nb = int(os.environ.get("GAT_BUFS", "2"))
                _nwl = int(os.environ.get("GAT_NWLIM", str(NW)))
                GMAX = int(os.environ.get("GAT_GMAX", "4"))
                idx_lo = idx_lo1 if layer == 1 else idx_lo2
                idx_hi = idx_hi1 if layer == 1 else idx_hi2
                tf_ev = tfull[:, :].rearrange("(n two) r -> n (two r)",
                                              two=2)[:, 0:R]
                tf_od = tfull[:, :].rearrange("(n two) r -> n (two r)",
                                              two=2)[:, R:2 * R]
                with tc.tile_pool(name=f"pe{sfx}", bufs=_nb) as pe, \
                     tc.tile_pool(name=f"peps{sfx}", bufs=2, space="PSUM") as pps, \
                     tc.tile_pool(name=f"po{sfx}", bufs=_nb) as po, \
                     tc.tile_pool(name=f"pops{sfx}", bufs=2, space="PSUM") as ops:
                    for w in range(min(NW, _nwl)):
                        ilo = pe.tile([128, KL * 8], I16, tag="ilo")
                        nc.sync.dma_start(
                            ilo[:], idx_lo[:, w * KL * 8:(w + 1) * KL * 8])
                        ihi = pe.tile([128, KH * 8], I16, tag="ihi")
                        nc.sync.dma_start(
                            ihi[:], idx_hi[:, w * KH * 8:(w + 1) * KH * 8])
                        Ssb = pe.tile([128, K * 128], BF16, tag="Ssb")
                        nc.sync.dma_start(
                            Ssb[:], Sg[:, w * K * 128:(w + 1) * K * 128])
                        STsb = pe.tile([128, K * 128], BF16, tag="STsb")
                        nc.sync.dma_start(
                            STsb[:], SgT[:, w * K * 128:(w + 1) * K * 128])
                        edw = pe.tile([128, H], BF16, tag="edw")
                        nc.sync.dma_start(edw[:],
                                          edst[w * WIN:(w + 1) * WIN, :])

                        glo = pe.tile([128, KL * R], BF16, tag="glo")
                        ghi = pe.tile([128, KH * R], BF16, tag="ghi")

                        def gcalls(tile_, src_ap, idx_tile, nblk, elem):
                            gv = tile_[:].rearrange("p (t e) -> p t e",
                                                    e=elem)
                            for g0 in range(0, nblk, GMAX):
                                nb = min(GMAX, nblk - g0)
                                nc.gpsimd.dma_gather(
                                    gv[:, g0:g0 + nb, :], src_ap,
                                    idx_tile[:, g0 * 8:(g0 + nb) * 8],
                                    nb * WIN, nb * WIN, elem,
                                    elem_step=2 * elem,
                                    queue_num=qn[0] % _nq)
                                qn[0] += 1

                        gcalls(glo, tf_ev, ilo, KL, R)
                        gcalls(ghi, tf_od, ihi, KH, R)

                        # per-edge e_dst via K tiny matmuls (S_T stationary)
                        psE = pps.tile([128, K * H], F32, tag="psE")
                        for b in range(K):
                            nc.tensor.matmul(
                                psE[:, b * H:(b + 1) * H],
                                STsb[:, b * 128:(b + 1) * 128], edw[:],
                                start=True, stop=True)

                        # embedded e_src -> compact.  Phase 1 is vector-bound
                        # (epilogue1 rides on vector), so both extracts go to
                        # scalar there; phase 2 splits them for balance.
                        es_c = pe.tile([128, K * H], F32, tag="es_c")
                        esv = es_c[:].rearrange("p (k h) -> p k h", h=H)
                        lo_eng = nc.scalar if layer == 1 else nc.vector
                        lo_src = (glo[:].rearrange("p (t e) -> p t e", e=R)
                                  [:, :, F:F + 8].bitcast(F32))
                        if layer == 1:
                            nc.scalar.copy(esv[:, 0:KL, :], lo_src)
                        else:
                            nc.vector.tensor_copy(esv[:, 0:KL, :], lo_src)
                        nc.scalar.copy(
                            esv[:, KL:K, :],
                            ghi[:].rearrange("p (t e) -> p t e", e=R)
                                [:, :, F:F + 8].bitcast(F32))

                        e_all = pe.tile([128, K * H], F32, tag="e_all")
                        nc.vector.tensor_tensor(e_all[:], es_c[:], psE[:],
                                                ALU.add)
                        e_sc = pe.tile([128, K * H], F32, tag="e_sc")
                        nc.vector.tensor_scalar(e_sc[:], e_all[:], NEG_SLOPE,
                                                None, ALU.mult)
                        nc.vector.tensor_tensor(e_all[:], e_all[:], e_sc[:],
                                                ALU.max)
                        w_all = pe.tile([128, K * H], BF16, tag="w_all")
                        nc.scalar.activation(w_all[:], e_all[:], AF.Exp)
                        # exp with broadcast input -> pre-expanded weights so
                        # the big multiply below is contiguous bf16 (2x DVE)
                        w_exp = pe.tile([128, K * F], BF16, tag="w_exp")
                        nc.scalar.activation(
                            w_exp[:].rearrange("p (k h c) -> p k h c", h=H,
                                               c=C),
                            e_all[:].rearrange("p (k h) -> p k h", h=H)
                                .unsqueeze(-1).broadcast_to([128, K, H, C]),
                            AF.Exp)

                        RC = F + H  # rhs cols per block
                        rhs = pe.tile([128, K * RC], BF16, tag="rhs")
                        rv = rhs[:].rearrange("p (k r) -> p k r", r=RC)
                        wv = w_all[:].rearrange("p (k h) -> p k h", h=H)
                        wev = w_exp[:].rearrange("p (k f) -> p k f", f=F)
                        nc.vector.tensor_tensor(
                            rv[:, 0:KL, 0:F],
                            glo[:].rearrange("p (t e) -> p t e", e=R)
                                [:, :, 0:F],
                            wev[:, 0:KL, :], ALU.mult)
                        nc.vector.tensor_tensor(
                            rv[:, KL:K, 0:F],
                            ghi[:].rearrange("p (t e) -> p t e", e=R)
                                [:, :, 0:F],
                            wev[:, KL:K, :], ALU.mult)
                        nc.vector.tensor_copy(rv[:, :, F:F + H], wv)

                        if layer == 1:
                            psW = pps.tile([128, F + H], F32, tag="psW")
                            for b in range(K):
                                nc.tensor.matmul(
                                    psW[:], Ssb[:, b * 128:(b + 1) * 128],
                                    rv[:, b, :], start=(b == 0),
                                    stop=(b == K - 1))
                            _epilogue1(nc, tc, po, ops, psW[:, 0:F1],
                                       psW[:, F1:F1 + H], bc, W2a,
                                       W2b, ident, cfg, w, t2_shard, edst2)
                            if "2" in PH and w in chunk_end:
                                emit_cc(t2_shard, t2_full, chunk_end[w], F2 + 8)
                        else:
                            psA = pps.tile([128, F2], F32, tag="psA")
                            psD = pps.tile([128, H], F32, tag="psD")
                            for b in range(K):
                                S_b = Ssb[:, b * 128:(b + 1) * 128]
                                nc.tensor.matmul(
                                    psA[:], S_b, rv[:, b, 0:F],
                                    start=(b == 0), stop=(b == K - 1))
                                nc.tensor.matmul(
                                    psD[:], S_b, rv[:, b, F:F + H],
                                    start=(b == 0), stop=(b == K - 1))
                            _epilogue2(nc, po, psA, psD, bc, cfg, w, out)

            def _epilogue1(nc, tc, po, ops, psA, psD, bc, W2a, W2b, ident,
                           cfg, w, t2_shard, edst2):
                den = po.tile([128, H], F32, tag="den")
                nc.vector.tensor_scalar(den[:], psD[:], 1e-30,
                                        None, ALU.max)
                rec = po.tile([128, H], F32, tag="rec")
                nc.vector.reciprocal(rec[:], den[:])
                o1 = po.tile([128, F1], F32, tag="o1")
                nc.vector.tensor_tensor(
                    o1[:].rearrange("p (h c) -> p h c", c=C1),
                    psA[:].rearrange("p (h c) -> p h c", c=C1),
                    rec[:].unsqueeze(-1).broadcast_to([128, H, C1]),
                    ALU.mult)
                nc.vector.tensor_tensor(o1[:], o1[:], bc["b1"][:], ALU.add)
                o1b = po.tile([128, F1], BF16, tag="o1b")
                nc.scalar.activation(o1b[:], o1[:], AF.Relu)
                tp = ops.tile([128, 128], BF16, tag="tp")
                t0 = po.tile([128, 128], BF16, tag="t0")
                nc.tensor.transpose(tp[:], o1b[:, 0:128], identb[:])
                nc.scalar.copy(t0[:], tp[:])
                tp2 = ops.tile([128, 128], BF16, tag="tp")
                t1t = po.tile([128, 128], BF16, tag="t1t")
                nc.tensor.transpose(tp2[:], o1b[:, 128:256], identb[:])
                nc.scalar.copy(t1t[:], tp2[:])
                h2 = ops.tile([128, F2], F32, tag="h2")
                nc.tensor.matmul(h2[:], t0[:], W2a[:], start=True, stop=False)
                nc.tensor.matmul(h2[:], t1t[:], W2b[:], start=False, stop=True)
                pk2 = po.tile([128, R2], BF16, tag="pk2")
                nc.scalar.copy(pk2[:, 0:F2], h2[:])
                h2b = pk2[:, 0:F2]
                pr = po.tile([128, F2], BF16, tag="pr")
                es2 = po.tile([128, H], F32, tag="es2")
                nc.vector.tensor_tensor(pr[:], h2b, bcb["asrc2"][:], ALU.mult)
                nc.vector.reduce_sum(
                    es2[:], pr[:].rearrange("p (h c) -> p h c", c=C2),
                    axis=mybir.AxisListType.X)
                pr2 = po.tile([128, F2], BF16, tag="pr2")
                ed2f = po.tile([128, H], F32, tag="ed2f")
                ed2 = po.tile([128, H], BF16, tag="ed2")
                nc.vector.tensor_tensor(pr2[:], h2b, bcb["adst2"][:],
                                        ALU.mult)
                nc.vector.reduce_sum(
                    ed2f[:], pr2[:].rearrange("p (h c) -> p h c", c=C2),
                    axis=mybir.AxisListType.X)
                nc.scalar.copy(ed2[:], ed2f[:])
                nc.vector.tensor_copy(pk2[:, F2:F2 + 8].bitcast(F32), es2[:])
                nc.sync.dma_start(t2_shard[w * WIN:(w + 1) * WIN, 0:F2 + 8],
                                  pk2[:, 0:F2 + 8])
                nc.sync.dma_start(edst2[w * WIN:(w + 1) * WIN, :], ed2[:])

            def _epilogue2(nc, po, psA, psD, bc, cfg, w, out):
                den = po.tile([128, H], F32, tag="den2")
                nc.vector.tensor_scalar(den[:], psD[:], 1e-30, None, ALU.max)
                rec = po.tile([128, H], F32, tag="rec2")
                nc.vector.reciprocal(rec[:], den[:])
                tmp = po.tile([128, F2], F32, tag="tmp")
                nc.vector.tensor_tensor(
                    tmp[:].rearrange("p (h c) -> p h c", c=C2),
                    psA[:].rearrange("p (h c) -> p h c", c=C2),
                    rec[:].unsqueeze(-1).broadcast_to([128, H, C2]),
                    ALU.mult)
                s01 = po.tile([128, C2], F32, tag="s01")
                nc.vector.tensor_tensor(s01[:], tmp[:, 0:C2],
                                        tmp[:, C2:2 * C2], ALU.add)
                s23 = po.tile([128, C2], F32, tag="s23")
                nc.vector.tensor_tensor(s23[:], tmp[:, 2 * C2:3 * C2],
                                        tmp[:, 3 * C2:4 * C2], ALU.add)
                o2 = po.tile([128, C2], F32, tag="o2")
                nc.vector.tensor_tensor(o2[:], s01[:], s23[:], ALU.add)
                nc.vector.tensor_scalar(o2[:], o2[:], 1.0 / H, None, ALU.mult)
                nc.vector.tensor_tensor(o2[:], o2[:], bc["b2"][:], ALU.add)
                # LayerNorm over C2
                mu = po.tile([128, 1], F32, tag="mu")
                nc.vector.reduce_sum(mu[:], o2[:], axis=mybir.AxisListType.X)
                nc.vector.tensor_scalar(mu[:], mu[:], 1.0 / C2, None, ALU.mult)
                xc = po.tile([128, C2], F32, tag="xc")
                nc.vector.tensor_scalar(xc[:], o2[:], mu[:], None,
                                        ALU.subtract)
                sq = po.tile([128, C2], F32, tag="sq")
                ssq = po.tile([128, 1], F32, tag="ssq")
                nc.scalar.activation(sq[:], xc[:], AF.Square, accum_out=ssq[:])
                sdev = po.tile([128, 1], F32, tag="sdev")
                nc.scalar.activation(sdev[:], ssq[:], AF.Sqrt,
                                     scale=1.0 / C2, bias=epsb[:, 0:1])
                rstd = po.tile([128, 1], F32, tag="rstd")
                nc.vector.reciprocal(rstd[:], sdev[:])
                xn = po.tile([128, C2], F32, tag="xn")
                nc.vector.tensor_scalar(xn[:], xc[:], rstd[:], None, ALU.mult)
                nc.vector.tensor_tensor(xn[:], xn[:], bc["gamma"][:],
                                        ALU.mult)
                ot = po.tile([128, C2], F32, tag="ot")
                nc.vector.tensor_tensor(ot[:], xn[:], bc["beta"][:], ALU.add)
                nc.sync.dma_start(out[w * WIN:(w + 1) * WIN, :], ot[:])

            if "1" in PH:
                edge_phase(1)
            if "2" in PH:
                edge_phase(2)

    nc.finalize()
    return nc


# --------------------------------------------------------------------------
_CACHE = {}


def run(inputs, trace=False):
    cfg, in_maps, slot_global = prep(**inputs)
    ckey = (cfg.key, os.environ.get("GAT_PHASES", "012"),
            os.environ.get("GAT_BUFS", "2"), os.environ.get("GAT_NWLIM"),
            os.environ.get("GAT_SCRATCH"), os.environ.get("GAT_GMAX"),
            os.environ.get("GAT_QUEUES"), os.environ.get("GAT_CHUNKS"))
    nc = _CACHE.get(ckey)
    if nc is None:
        nc = build(cfg)
        _CACHE[ckey] = nc
    if os.environ.get("GAT_SIM"):
        from concourse.bass_interp import MultiCoreSim
        nc.insert_bir_kernel_barrier_sem_inc()
        sim = MultiCoreSim(nc, CORES, aliases={}, require_finite=False,
                           require_nnan=False)
        innames = [a.memorylocations[0].name
                   for a in nc.m.functions[0].allocations
                   if getattr(a, 'kind', None) == 'ExternalInput']
        for t in range(CORES):
            for nm in innames:
                if nm == 'partition_id':
                    sim.cores[t].tensor(nm)[:] = np.array([[t]], np.uint32)
                else:
                    sim.cores[t].tensor(nm)[:] = in_maps[t][nm]
        sim.simulate()
        class R: pass
        res = R(); res.results = [
            {"out": np.array(sim.cores[t].tensor("out"))} for t in range(CORES)]
        res.exec_time_ns = None
    else:
        res = run_bass_kernel_spmd(nc, in_maps, core_ids=list(range(CORES)),
                                   trace=trace)
    full = np.concatenate([res.results[c]["out"] for c in range(CORES)],
                          axis=0)
    return full[slot_global], res


# --------------------------------------------------------------------------
# harness entry point
# --------------------------------------------------------------------------

def kernel(**inputs):
    """Full unsharded inputs -> full [N, 128] output (runs on 8 NeuronCores)."""
    out, _ = run(inputs)
    return out


# revision 49
# speedup vs baseline: 1.0235x; 1.0235x over previous
"""GATConv x2 + LayerNorm (GNN message passing) on 8 TRN2 NeuronCores.

Strategy (edge-parallel, dst-sharded):
  - Nodes are sharded across 8 cores by id range; each core owns all edges
    whose dst falls in its range (plus self loops).
  - Host assigns each core's nodes to 128-slot "windows", balancing total
    in-degree per window; edges are laid out per window in 128-edge blocks,
    split into low/high halves by source table row (int16 gather indices).
  - Phase 0 (per core): h1 = x @ W1 for own nodes, attention dot products
    e_src1/e_dst1; packed node table rows [h1 | e_src1] -> AllGather.
  - Phase 1: per window, dma_gather source rows (round-robin over 4 SWDGE
    queues so descriptor generation overlaps across Q7 core pairs), load
    host-precomputed one-hot matrices S (edge->dstslot) and S_T
    (dstslot->edge), broadcast per-window e_dst to edges via K tiny
    matmuls with S_T stationary, extract embedded e_src with the scalar
    engine, compute exp(leaky_relu(e_src+e_dst)) in bf16, one matmul per
    128-edge block with S stationary accumulates the weighted message sum
    and softmax denominators in PSUM.  Window epilogue normalizes, applies
    relu, computes h2 = relu(out1) @ W2 and the layer-2 attention dots;
    packed table 2 -> AllGather.
  - Phase 2: same edge machinery on table 2; epilogue does head-mean,
    bias, LayerNorm, and writes the output rows (window-slot order; host
    unpermutes).

Tables store h in bf16 with e-values embedded as f32 (bitcast slices).
"""

import sys

sys.path.insert(0, "/opt/trn_rl_repo")

import math
import os
import numpy as np
import ml_dtypes

import concourse.bass as bass
import concourse.bacc as bacc
import concourse.mybir as mybir
from concourse import tile
from concourse.bass_utils import run_bass_kernel_spmd

F32 = mybir.dt.float32
BF16 = mybir.dt.bfloat16
I16 = mybir.dt.int16
I32 = mybir.dt.int32
AF = mybir.ActivationFunctionType
ALU = mybir.AluOpType

CORES = 8
WIN = 128
H = 4

NEG_SLOPE = 0.2
EPS_LN = 1e-5


class Cfg:
    def __init__(self, N, IN_DIM, C1, C2, KL, KH, CH):
        assert N % CORES == 0
        self.N = N
        self.IN = IN_DIM          # input feature dim (=128)
        self.C1 = C1              # per-head dim layer 1 (64)
        self.C2 = C2              # per-head dim layer 2 (128)
        self.F1 = H * C1          # 256
        self.F2 = H * C2          # 512
        self.NODES_PC = N // CORES
        self.NW = math.ceil(self.NODES_PC / WIN)
        self.SLOTS_PC = self.NW * WIN
        self.TOT = CORES * self.SLOTS_PC
        assert self.TOT % 2 == 0
        self.HALF = self.TOT // 2
        assert self.HALF <= 32767, self.HALF
        assert self.SLOTS_PC <= 32767
        self.KL = KL
        self.KH = KH
        self.K = KL + KH
        # chunked AllGather: table rows are chunk-major
        # row(core,w,o) = crb[chunk(w)] + core*cw[chunk]*WIN + (w-w0)*WIN + o
        self.CH = CH
        self.cw = [self.NW // CH + (1 if i < self.NW % CH else 0)
                   for i in range(CH)]
        self.w0s = np.concatenate([[0], np.cumsum(self.cw)])
        self.crb = np.concatenate(
            [[0], np.cumsum([CORES * c * WIN for c in self.cw])])[:-1]
        # packed table rows (bf16 elements, 256B-multiple bytes)
        self.R1 = _pad_row(self.F1 * 2 + 16)   # bf16 cols
        self.R2 = _pad_row(self.F2 * 2 + 16)
        self.key = (N, IN_DIM, C1, C2, KL, KH, CH)

    def grow(self, core, w, o):
        """Global table row for (core, window, offset) — vectorized."""
        w = np.asarray(w)
        ch = np.searchsorted(self.w0s, w, side="right") - 1
        cw = np.asarray(self.cw)[ch]
        w0 = self.w0s[ch]
        return self.crb[ch] + core * cw * WIN + (w - w0) * WIN + o


def _pad_row(nbytes):
    """Round row bytes up to a multiple of 256; return bf16 col count."""
    b = ((nbytes + 255) // 256) * 256
    return b // 2


def _wrap_idx(arr):
    """[NWxL] int -> [128, NW*L/16] int16 wrapped+replicated per call."""
    nw, L = arr.shape
    assert L % 16 == 0
    w = arr.reshape(nw, L // 16, 16).transpose(0, 2, 1)  # [nw, 16, L/16]
    w = np.concatenate([w] * 8, axis=1)                  # [nw, 128, L/16]
    w = np.concatenate(list(w), axis=1)                  # [128, nw*L/16]
    return np.ascontiguousarray(w.astype(np.int16))


def prep(x, edge_index, W1, a_src1, a_dst1, b1, W2, a_src2, a_dst2, b2,
         gamma, beta):
    """Host-side sharding. Returns (cfg, in_maps, slot_global)."""
    N, IN_DIM = x.shape
    C1 = a_src1.shape[1]
    C2 = a_src2.shape[1]

    src = np.asarray(edge_index[0], dtype=np.int64)
    dst = np.asarray(edge_index[1], dtype=np.int64)
    loop = np.arange(N, dtype=np.int64)
    src = np.concatenate([src, loop])
    dst = np.concatenate([dst, loop])

    NODES_PC = N // CORES
    NW = math.ceil(NODES_PC / WIN)
    SLOTS_PC = NW * WIN

    # ---- window assignment per core (balance in-degree across NW bins) ----
    deg = np.bincount(dst, minlength=N)
    slot_global = np.empty(N, dtype=np.int64)
    win_of = np.empty(N, dtype=np.int64)    # window within core
    off_of = np.empty(N, dtype=np.int64)    # slot within window
    for c in range(CORES):
        nodes = np.arange(c * NODES_PC, (c + 1) * NODES_PC)
        d = deg[nodes]
        order = np.argsort(-d, kind="stable")
        # greedy: place next-heaviest node into least-loaded non-full bin
        bin_load = np.zeros(NW, dtype=np.int64)
        bin_cnt = np.zeros(NW, dtype=np.int64)
        wsel = np.empty(len(nodes), dtype=np.int64)
        osel = np.empty(len(nodes), dtype=np.int64)
        import heapq
        heap = [(0, 0, w) for w in range(NW)]
        heapq.heapify(heap)
        for i in order:
            while True:
                load, cnt, w = heapq.heappop(heap)
                if cnt < WIN:
                    break
            wsel[i] = w
            osel[i] = cnt
            heapq.heappush(heap, (load + d[i], cnt + 1, w))
        win_of[nodes] = wsel
        off_of[nodes] = osel
        slot_global[nodes] = c * SLOTS_PC + wsel * WIN + osel

    HALF = CORES * SLOTS_PC // 2
    CH = int(os.environ.get("GAT_CHUNKS", "4"))
    cfg0 = Cfg(N, IN_DIM, C1, C2, 0, 0, CH)

    owner = dst // NODES_PC
    src_core = src // NODES_PC

    # ---- pass 2: rebalance bins on (lo, hi) in-degree jointly ----
    # lo/hi labels from the pass-1 placement (approximation; final KL/KH
    # are recomputed from the final placement below)
    lo_lbl = (off_of[src] % 2) == 0
    dlo = np.bincount(dst[lo_lbl], minlength=N)
    dhi = np.bincount(dst[~lo_lbl], minlength=N)
    for c in range(CORES):
        nodes = np.arange(c * NODES_PC, (c + 1) * NODES_PC)
        dl, dh = dlo[nodes], dhi[nodes]
        order = np.argsort(-(dl + dh), kind="stable")
        binlo = np.zeros(NW, dtype=np.int64)
        binhi = np.zeros(NW, dtype=np.int64)
        bincnt = np.zeros(NW, dtype=np.int64)
        for i in order:
            cost = np.maximum(binlo + dl[i], binhi + dh[i]).astype(np.float64)
            cost[bincnt >= WIN] = np.inf
            w = int(np.argmin(cost))
            win_of[nodes[i]] = w
            off_of[nodes[i]] = bincnt[w]
            binlo[w] += dl[i]
            binhi[w] += dh[i]
            bincnt[w] += 1
        slot_global[nodes] = (c * SLOTS_PC + win_of[nodes] * WIN
                              + off_of[nodes])

    # ---- per-core edge layout (final placement) ----
    # Gathers are split by slot-offset PARITY (even/odd), which is identical
    # under both the core-major (t1) and chunk-major (t2) row numberings:
    # row = base*128 + o in both, so parity(row) = parity(o).  Each stream
    # gathers with elem_step = 2 rows, so int16 indices (row//2) cover the
    # whole table.  "lo" = even, "hi" = odd below.
    r1_of = slot_global                                  # core-major rows
    r2_of = cfg0.grow(np.arange(N) // NODES_PC, win_of, off_of)
    src_r1 = r1_of[src]
    src_r2 = r2_of[src]
    e_w = win_of[dst]
    e_off = off_of[dst]
    e_low = (src_r1 % 2) == 0

    # first pass: find KL / KH
    KL = 0
    KH = 0
    per_core = []
    for c in range(CORES):
        m = owner == c
        ew, eo, el = e_w[m], e_off[m], e_low[m]
        er1, er2 = src_r1[m] // 2, src_r2[m] // 2
        nlo = np.bincount(ew[el], minlength=NW)
        nhi = np.bincount(ew[~el], minlength=NW)
        KL = max(KL, int(np.ceil(nlo.max() / WIN)))
        KH = max(KH, int(np.ceil(nhi.max() / WIN)))
        per_core.append((ew, eo, er1, er2, el))
    cfg = Cfg(N, IN_DIM, C1, C2, KL, KH, CH)
    K = cfg.K

    in_maps = []
    for c in range(CORES):
        ew, eo, er1, er2, el = per_core[c]
        idx_lo1 = np.zeros((NW, KL * WIN), dtype=np.int64)
        idx_hi1 = np.zeros((NW, KH * WIN), dtype=np.int64)
        idx_lo2 = np.zeros((NW, KL * WIN), dtype=np.int64)
        idx_hi2 = np.zeros((NW, KH * WIN), dtype=np.int64)
        dstoff = np.full((NW, K * WIN), 999, dtype=np.int64)
        for w in range(NW):
            wm = ew == w
            lo_m = wm & el
            hi_m = wm & ~el
            # ascending source rows within each stream -> DMA reads walk
            # HBM mostly monotonically (better row-buffer locality)
            plo = np.argsort(er1[lo_m], kind="stable")
            phi = np.argsort(er1[hi_m], kind="stable")
            olo, ohi = eo[lo_m][plo], eo[hi_m][phi]
            idx_lo1[w, : len(olo)] = er1[lo_m][plo]
            idx_hi1[w, : len(ohi)] = er1[hi_m][phi]
            idx_lo2[w, : len(olo)] = er2[lo_m][plo]
            idx_hi2[w, : len(ohi)] = er2[hi_m][phi]
            dstoff[w, : len(olo)] = olo
            dstoff[w, KL * WIN : KL * WIN + len(ohi)] = ohi

        # one-hot matrices, [128, NW*K*128] bf16:
        #   Sg [e, (w,b,s)] = (dstoff[(w,b,e)] == s)   (edge-partition)
        #   SgT[s, (w,b,e)] = same                      (dstslot-partition)
        flat = dstoff.reshape(-1)                    # [(w,b,e)]
        j = np.arange(flat.size)
        valid = flat < WIN
        jv, fv = j[valid], flat[valid]
        L = NW * K * WIN
        Sg = np.zeros((WIN, L), dtype=ml_dtypes.bfloat16)
        Sg[jv % WIN, (jv // WIN) * WIN + fv] = 1
        SgT = np.zeros((WIN, L), dtype=ml_dtypes.bfloat16)
        SgT[fv, jv] = 1

        xs = np.zeros((SLOTS_PC, IN_DIM), dtype=np.float32)
        nodes = np.arange(c * NODES_PC, (c + 1) * NODES_PC)
        loc = win_of[nodes] * WIN + off_of[nodes]
        xs[loc] = np.asarray(x[nodes], dtype=np.float32)

        in_maps.append({
            "xT": np.ascontiguousarray(xs.T),
            "idx_lo1": _wrap_idx(idx_lo1),
            "idx_hi1": _wrap_idx(idx_hi1),
            "idx_lo2": _wrap_idx(idx_lo2),
            "idx_hi2": _wrap_idx(idx_hi2),
            "Sg": np.ascontiguousarray(Sg),
            "SgT": np.ascontiguousarray(SgT),
            "W1": np.asarray(W1, np.float32),
            "W2": np.asarray(W2, np.float32),
            "asrc1": np.asarray(a_src1, np.float32).reshape(1, -1),
            "adst1": np.asarray(a_dst1, np.float32).reshape(1, -1),
            "asrc2": np.asarray(a_src2, np.float32).reshape(1, -1),
            "adst2": np.asarray(a_dst2, np.float32).reshape(1, -1),
            "b1": np.asarray(b1, np.float32).reshape(1, -1),
            "b2": np.asarray(b2, np.float32).reshape(1, -1),
            "gamma": np.asarray(gamma, np.float32).reshape(1, -1),
            "beta": np.asarray(beta, np.float32).reshape(1, -1),
        })
    return cfg, in_maps, slot_global


# --------------------------------------------------------------------------
# device program
# --------------------------------------------------------------------------

def build(cfg):
    PH = os.environ.get("GAT_PHASES", "012")
    _sc = int(os.environ.get("GAT_SCRATCH", "16384"))
    _nq = int(os.environ.get("GAT_QUEUES", "4"))
    nc = bacc.Bacc("TRN2", target_bir_lowering=False, debug=False,
                   num_devices=CORES, dynamic_dma_scratch_size=_sc,
                   num_swdge_queues=_nq)
    NW, K, KL, KH = cfg.NW, cfg.K, cfg.KL, cfg.KH
    F1, F2, R1, R2 = cfg.F1, cfg.F2, cfg.R1, cfg.R2
    C1, C2 = cfg.C1, cfg.C2
    SL, TOT, HALF = cfg.SLOTS_PC, cfg.TOT, cfg.HALF
    RG = [list(range(CORES))]
    CH, cw, w0s, crb = cfg.CH, cfg.cw, cfg.w0s, cfg.crb
    chunk_end = {int(w0s[i + 1]) - 1: i for i in range(CH)}

    def emit_cc(shard, full, ch, ccc):
        w0, w1 = int(w0s[ch]), int(w0s[ch + 1])
        b = int(crb[ch])
        nc.gpsimd.collective_compute(
            "AllGather", ALU.bypass, replica_groups=RG,
            ins=[shard[w0 * WIN:w1 * WIN, :]],
            outs=[full[b:b + CORES * (w1 - w0) * WIN, :]])

    # ---- kernel I/O ----
    xT = nc.dram_tensor("xT", [cfg.IN, SL], F32, kind="ExternalInput")
    idx_lo1 = nc.dram_tensor("idx_lo1", [128, NW * KL * 8], I16, kind="ExternalInput")
    idx_hi1 = nc.dram_tensor("idx_hi1", [128, NW * KH * 8], I16, kind="ExternalInput")
    idx_lo2 = nc.dram_tensor("idx_lo2", [128, NW * KL * 8], I16, kind="ExternalInput")
    idx_hi2 = nc.dram_tensor("idx_hi2", [128, NW * KH * 8], I16, kind="ExternalInput")
    Sg = nc.dram_tensor("Sg", [128, NW * K * 128], BF16, kind="ExternalInput")
    SgT = nc.dram_tensor("SgT", [128, NW * K * 128], BF16, kind="ExternalInput")
    W1 = nc.dram_tensor("W1", [cfg.IN, F1], F32, kind="ExternalInput")
    W2 = nc.dram_tensor("W2", [F1, F2], F32, kind="ExternalInput")
    vecs = {}
    for nm, d in [("asrc1", F1), ("adst1", F1), ("asrc2", F2), ("adst2", F2),
                  ("b1", F1), ("b2", C2), ("gamma", C2), ("beta", C2)]:
        vecs[nm] = nc.dram_tensor(nm, [1, d], F32, kind="ExternalInput")
    out = nc.dram_tensor("out", [SL, C2], F32, kind="ExternalOutput")

    # ---- internal DRAM ----
    t1_shard = nc.dram_tensor("t1_shard", [SL, R1], BF16, kind="Internal")
    t2_shard = nc.dram_tensor("t2_shard", [SL, R2], BF16, kind="Internal")
    t1_full = nc.dram_tensor("t1_full", [TOT, R1], BF16, kind="Internal",
                             addr_space="Shared")
    t2_full = nc.dram_tensor("t2_full", [TOT, R2], BF16, kind="Internal",
                             addr_space="Shared")
    edst1 = nc.dram_tensor("edst1", [SL, H], BF16, kind="Internal")
    edst2 = nc.dram_tensor("edst2", [SL, H], BF16, kind="Internal")

    with tile.TileContext(nc) as tc:
        with tc.tile_pool(name="const", bufs=1) as cp:
            iota_i = cp.tile([128, 128], I32)
            nc.gpsimd.iota(iota_i[:], pattern=[[1, 128]], base=0,
                           channel_multiplier=0)
            iota_f = cp.tile([128, 128], F32)
            nc.vector.tensor_copy(iota_f[:], iota_i[:])
            ic_i = cp.tile([128, 1], I32)
            nc.gpsimd.iota(ic_i[:], pattern=[[0, 1]], base=0,
                           channel_multiplier=1)
            ic_f = cp.tile([128, 1], F32)
            nc.vector.tensor_copy(ic_f[:], ic_i[:])
            ident = cp.tile([128, 128], F32)
            nc.vector.tensor_scalar(ident[:], iota_f[:], ic_f[:, 0:1],
                                    None, ALU.is_equal)

            W1sb = cp.tile([128, F1], F32)
            nc.sync.dma_start(W1sb[:], W1[:, :])
            W2af = cp.tile([128, F2], F32)
            W2bf = cp.tile([128, F2], F32)
            nc.sync.dma_start(W2af[:], W2[0:128, :])
            nc.sync.dma_start(W2bf[:], W2[128:256, :])
            W2a = cp.tile([128, F2], BF16)
            W2b = cp.tile([128, F2], BF16)
            nc.vector.tensor_copy(W2a[:], W2af[:])
            nc.vector.tensor_copy(W2b[:], W2bf[:])
            identb = cp.tile([128, 128], BF16)
            nc.vector.tensor_copy(identb[:], ident[:])
            bcb = {}

            ones = cp.tile([1, 128], F32)
            nc.vector.memset(ones[:], 1.0)
            epsb = cp.tile([128, 1], F32)
            nc.vector.memset(epsb[:], EPS_LN)

            # broadcast small vectors to [128, D] via 1-row matmul
            bc = {}
            with tc.tile_pool(name="bcp", bufs=2, space="PSUM") as bps, \
                 tc.tile_pool(name="bcs", bufs=1) as bsb:
                for nm, d in [("asrc1", F1), ("adst1", F1), ("asrc2", F2),
                              ("adst2", F2), ("b1", F1), ("b2", C2),
                              ("gamma", C2), ("beta", C2)]:
                    vsb = bsb.tile([1, d], F32, tag="vload")
                    nc.sync.dma_start(vsb[:], vecs[nm][:, :])
                    t = cp.tile([128, d], F32, tag=f"bc_{nm}")
                    ps = bps.tile([128, d], F32, tag="bcps")
                    nc.tensor.matmul(ps[:], ones[:], vsb[:], start=True,
                                     stop=True)
                    nc.vector.tensor_copy(t[:], ps[:])
                    bc[nm] = t
                    if nm in ("asrc1", "adst1", "asrc2", "adst2"):
                        tb = cp.tile([128, d], BF16, tag=f"bcb_{nm}")
                        nc.vector.tensor_copy(tb[:], ps[:])
                        bcb[nm] = tb

            # ================= Phase 0: node tables =================
            with tc.tile_pool(name="p0", bufs=3) as p0, \
                 tc.tile_pool(name="p0ps", bufs=2, space="PSUM") as p0ps:
                for w in range(NW):
                    xt = p0.tile([128, 128], F32, tag="xt")
                    nc.sync.dma_start(xt[:], xT[:, w * WIN:(w + 1) * WIN])
                    h1 = p0ps.tile([128, F1], F32, tag="h1")
                    nc.tensor.matmul(h1[:], xt[:], W1sb[:], start=True,
                                     stop=True)
                    pk = p0.tile([128, R1], BF16, tag="pk")
                    nc.scalar.copy(pk[:, 0:F1], h1[:])
                    h1b = pk[:, 0:F1]
                    prod = p0.tile([128, F1], BF16, tag="prod")
                    es1 = p0.tile([128, H], F32, tag="es1")
                    nc.vector.tensor_tensor(prod[:], h1b, bcb["asrc1"][:],
                                            ALU.mult)
                    nc.vector.reduce_sum(
                        es1[:], prod[:].rearrange("p (h c) -> p h c", c=C1),
                        axis=mybir.AxisListType.X)
                    prod2 = p0.tile([128, F1], BF16, tag="prod2")
                    ed1f = p0.tile([128, H], F32, tag="ed1f")
                    ed1 = p0.tile([128, H], BF16, tag="ed1")
                    nc.vector.tensor_tensor(prod2[:], h1b, bcb["adst1"][:],
                                            ALU.mult)
                    nc.vector.reduce_sum(
                        ed1f[:], prod2[:].rearrange("p (h c) -> p h c", c=C1),
                        axis=mybir.AxisListType.X)
                    nc.vector.tensor_copy(ed1[:], ed1f[:])
                    nc.vector.tensor_copy(
                        pk[:, F1:F1 + 8].bitcast(F32), es1[:])
                    nc.sync.dma_start(
                        t1_shard[w * WIN:(w + 1) * WIN, 0:F1 + 8],
                        pk[:, 0:F1 + 8])
                    nc.sync.dma_start(edst1[w * WIN:(w + 1) * WIN, :],
                                      ed1[:])
            if "1" in PH or "2" in PH:
                # t1 is core-major: one fast AllGather (plain concat)
                nc.gpsimd.collective_compute(
                    "AllGather", ALU.bypass, replica_groups=RG,
                    ins=[t1_shard[:, :]], outs=[t1_full[:, :]])

            # ================= Phase 1 and 2 =================
            qn = [0]

            def edge_phase(layer):
                F = F1 if layer == 1 else F2
                C = C1 if layer == 1 else C2
                R = R1 if layer == 1 else R2
                tfull = t1_full if layer == 1 else t2_full
                edst = edst1 if layer == 1 else edst2
                sfx = f"L{layer}"
                _nb = int(os.environ.get("GAT_BUFS", "2"))
                _nwl = int(os.environ.get("GAT_NWLIM", str(NW)))
                GMAX = int(os.environ.get("GAT_GMAX", "4"))
                idx_lo = idx_lo1 if layer == 1 else idx_lo2
                idx_hi = idx_hi1 if layer == 1 else idx_hi2
                tf_ev = tfull[:, :].rearrange("(n two) r -> n (two r)",
                                              two=2)[:, 0:R]
                tf_od = tfull[:, :].rearrange("(n two) r -> n (two r)",
                                              two=2)[:, R:2 * R]
                with tc.tile_pool(name=f"pe{sfx}", bufs=_nb) as pe, \
                     tc.tile_pool(name=f"peps{sfx}", bufs=2, space="PSUM") as pps, \
                     tc.tile_pool(name=f"po{sfx}", bufs=_nb) as po, \
                     tc.tile_pool(name=f"pops{sfx}", bufs=2, space="PSUM") as ops:
                    for w in range(min(NW, _nwl)):
                        ilo = pe.tile([128, KL * 8], I16, tag="ilo")
                        nc.sync.dma_start(
                            ilo[:], idx_lo[:, w * KL * 8:(w + 1) * KL * 8])
                        ihi = pe.tile([128, KH * 8], I16, tag="ihi")
                        nc.sync.dma_start(
                            ihi[:], idx_hi[:, w * KH * 8:(w + 1) * KH * 8])
                        Ssb = pe.tile([128, K * 128], BF16, tag="Ssb")
                        nc.sync.dma_start(
                            Ssb[:], Sg[:, w * K * 128:(w + 1) * K * 128])
                        STsb = pe.tile([128, K * 128], BF16, tag="STsb")
                        nc.sync.dma_start(
                            STsb[:], SgT[:, w * K * 128:(w + 1) * K * 128])
                        edw = pe.tile([128, H], BF16, tag="edw")
                        nc.sync.dma_start(edw[:],
                                          edst[w * WIN:(w + 1) * WIN, :])

                        glo = pe.tile([128, KL * R], BF16, tag="glo")
                        ghi = pe.tile([128, KH * R], BF16, tag="ghi")

                        def gcalls(tile_, src_ap, idx_tile, nblk, elem):
                            gv = tile_[:].rearrange("p (t e) -> p t e",
                                                    e=elem)
                            for g0 in range(0, nblk, GMAX):
                                nb = min(GMAX, nblk - g0)
                                nc.gpsimd.dma_gather(
                                    gv[:, g0:g0 + nb, :], src_ap,
                                    idx_tile[:, g0 * 8:(g0 + nb) * 8],
                                    nb * WIN, nb * WIN, elem,
                                    elem_step=2 * elem,
                                    queue_num=qn[0] % _nq)
                                qn[0] += 1

                        gcalls(glo, tf_ev, ilo, KL, R)
                        gcalls(ghi, tf_od, ihi, KH, R)

                        # per-edge e_dst via K tiny matmuls (S_T stationary)
                        psE = pps.tile([128, K * H], F32, tag="psE")
                        for b in range(K):
                            nc.tensor.matmul(
                                psE[:, b * H:(b + 1) * H],
                                STsb[:, b * 128:(b + 1) * 128], edw[:],
                                start=True, stop=True)

                        # embedded e_src -> compact (split vector/scalar)
                        es_c = pe.tile([128, K * H], F32, tag="es_c")
                        esv = es_c[:].rearrange("p (k h) -> p k h", h=H)
                        nc.vector.tensor_copy(
                            esv[:, 0:KL, :],
                            glo[:].rearrange("p (t e) -> p t e", e=R)
                                [:, :, F:F + 8].bitcast(F32))
                        nc.scalar.copy(
                            esv[:, KL:K, :],
                            ghi[:].rearrange("p (t e) -> p t e", e=R)
                                [:, :, F:F + 8].bitcast(F32))

                        e_all = pe.tile([128, K * H], F32, tag="e_all")
                        nc.vector.tensor_tensor(e_all[:], es_c[:], psE[:],
                                                ALU.add)
                        e_sc = pe.tile([128, K * H], F32, tag="e_sc")
                        nc.vector.tensor_scalar(e_sc[:], e_all[:], NEG_SLOPE,
                                                None, ALU.mult)
                        nc.vector.tensor_tensor(e_all[:], e_all[:], e_sc[:],
                                                ALU.max)
                        w_all = pe.tile([128, K * H], BF16, tag="w_all")
                        nc.scalar.activation(w_all[:], e_all[:], AF.Exp)
                        # exp with broadcast input -> pre-expanded weights so
                        # the big multiply below is contiguous bf16 (2x DVE)
                        w_exp = pe.tile([128, K * F], BF16, tag="w_exp")
                        nc.scalar.activation(
                            w_exp[:].rearrange("p (k h c) -> p k h c", h=H,
                                               c=C),
                            e_all[:].rearrange("p (k h) -> p k h", h=H)
                                .unsqueeze(-1).broadcast_to([128, K, H, C]),
                            AF.Exp)

                        RC = F + H  # rhs cols per block
                        rhs = pe.tile([128, K * RC], BF16, tag="rhs")
                        rv = rhs[:].rearrange("p (k r) -> p k r", r=RC)
                        wv = w_all[:].rearrange("p (k h) -> p k h", h=H)
                        wev = w_exp[:].rearrange("p (k f) -> p k f", f=F)
                        nc.vector.tensor_tensor(
                            rv[:, 0:KL, 0:F],
                            glo[:].rearrange("p (t e) -> p t e", e=R)
                                [:, :, 0:F],
                            wev[:, 0:KL, :], ALU.mult)
                        nc.vector.tensor_tensor(
                            rv[:, KL:K, 0:F],
                            ghi[:].rearrange("p (t e) -> p t e", e=R)
                                [:, :, 0:F],
                            wev[:, KL:K, :], ALU.mult)
                        nc.vector.tensor_copy(rv[:, :, F:F + H], wv)

                        if layer == 1:
                            psW = pps.tile([128, F + H], F32, tag="psW")
                            for b in range(K):
                                nc.tensor.matmul(
                                    psW[:], Ssb[:, b * 128:(b + 1) * 128],
                                    rv[:, b, :], start=(b == 0),
                                    stop=(b == K - 1))
                            _epilogue1(nc, tc, po, ops, psW[:, 0:F1],
                                       psW[:, F1:F1 + H], bc, W2a,
                                       W2b, ident, cfg, w, t2_shard, edst2)
                            if "2" in PH and w in chunk_end:
                                emit_cc(t2_shard, t2_full, chunk_end[w], F2 + 8)
                        else:
                            psA = pps.tile([128, F2], F32, tag="psA")
                            psD = pps.tile([128, H], F32, tag="psD")
                            for b in range(K):
                                S_b = Ssb[:, b * 128:(b + 1) * 128]
                                nc.tensor.matmul(
                                    psA[:], S_b, rv[:, b, 0:F],
                                    start=(b == 0), stop=(b == K - 1))
                                nc.tensor.matmul(
                                    psD[:], S_b, rv[:, b, F:F + H],
                                    start=(b == 0), stop=(b == K - 1))
                            _epilogue2(nc, po, psA, psD, bc, cfg, w, out)

            def _epilogue1(nc, tc, po, ops, psA, psD, bc, W2a, W2b, ident,
                           cfg, w, t2_shard, edst2):
                den = po.tile([128, H], F32, tag="den")
                nc.vector.tensor_scalar(den[:], psD[:], 1e-30,
                                        None, ALU.max)
                rec = po.tile([128, H], F32, tag="rec")
                nc.vector.reciprocal(rec[:], den[:])
                o1 = po.tile([128, F1], F32, tag="o1")
                nc.vector.tensor_tensor(
                    o1[:].rearrange("p (h c) -> p h c", c=C1),
                    psA[:].rearrange("p (h c) -> p h c", c=C1),
                    rec[:].unsqueeze(-1).broadcast_to([128, H, C1]),
                    ALU.mult)
                nc.vector.tensor_tensor(o1[:], o1[:], bc["b1"][:], ALU.add)
                o1b = po.tile([128, F1], BF16, tag="o1b")
                nc.scalar.activation(o1b[:], o1[:], AF.Relu)
                tp = ops.tile([128, 128], BF16, tag="tp")
                t0 = po.tile([128, 128], BF16, tag="t0")
                nc.tensor.transpose(tp[:], o1b[:, 0:128], identb[:])
                nc.vector.tensor_copy(t0[:], tp[:])
                tp2 = ops.tile([128, 128], BF16, tag="tp")
                t1t = po.tile([128, 128], BF16, tag="t1t")
                nc.tensor.transpose(tp2[:], o1b[:, 128:256], identb[:])
                nc.vector.tensor_copy(t1t[:], tp2[:])
                h2 = ops.tile([128, F2], F32, tag="h2")
                nc.tensor.matmul(h2[:], t0[:], W2a[:], start=True, stop=False)
                nc.tensor.matmul(h2[:], t1t[:], W2b[:], start=False, stop=True)
                pk2 = po.tile([128, R2], BF16, tag="pk2")
                nc.scalar.copy(pk2[:, 0:F2], h2[:])
                h2b = pk2[:, 0:F2]
                pr = po.tile([128, F2], BF16, tag="pr")
                es2 = po.tile([128, H], F32, tag="es2")
                nc.vector.tensor_tensor(pr[:], h2b, bcb["asrc2"][:], ALU.mult)
                nc.vector.reduce_sum(
                    es2[:], pr[:].rearrange("p (h c) -> p h c", c=C2),
                    axis=mybir.AxisListType.X)
                pr2 = po.tile([128, F2], BF16, tag="pr2")
                ed2f = po.tile([128, H], F32, tag="ed2f")
                ed2 = po.tile([128, H], BF16, tag="ed2")
                nc.vector.tensor_tensor(pr2[:], h2b, bcb["adst2"][:],
                                        ALU.mult)
                nc.vector.reduce_sum(
                    ed2f[:], pr2[:].rearrange("p (h c) -> p h c", c=C2),
                    axis=mybir.AxisListType.X)
                nc.vector.tensor_copy(ed2[:], ed2f[:])
                nc.vector.tensor_copy(pk2[:, F2:F2 + 8].bitcast(F32), es2[:])
                nc.sync.dma_start(t2_shard[w * WIN:(w + 1) * WIN, 0:F2 + 8],
                                  pk2[:, 0:F2 + 8])
                nc.sync.dma_start(edst2[w * WIN:(w + 1) * WIN, :], ed2[:])

            def _epilogue2(nc, po, psA, psD, bc, cfg, w, out):
                den = po.tile([128, H], F32, tag="den2")
                nc.vector.tensor_scalar(den[:], psD[:], 1e-30, None, ALU.max)
                rec = po.tile([128, H], F32, tag="rec2")
                nc.vector.reciprocal(rec[:], den[:])
                tmp = po.tile([128, F2], F32, tag="tmp")
                nc.vector.tensor_tensor(
                    tmp[:].rearrange("p (h c) -> p h c", c=C2),
                    psA[:].rearrange("p (h c) -> p h c", c=C2),
                    rec[:].unsqueeze(-1).broadcast_to([128, H, C2]),
                    ALU.mult)
                s01 = po.tile([128, C2], F32, tag="s01")
                nc.vector.tensor_tensor(s01[:], tmp[:, 0:C2],
                                        tmp[:, C2:2 * C2], ALU.add)
                s23 = po.tile([128, C2], F32, tag="s23")
                nc.vector.tensor_tensor(s23[:], tmp[:, 2 * C2:3 * C2],
                                        tmp[:, 3 * C2:4 * C2], ALU.add)
                o2 = po.tile([128, C2], F32, tag="o2")
                nc.vector.tensor_tensor(o2[:], s01[:], s23[:], ALU.add)
                nc.vector.tensor_scalar(o2[:], o2[:], 1.0 / H, None, ALU.mult)
                nc.vector.tensor_tensor(o2[:], o2[:], bc["b2"][:], ALU.add)
                # LayerNorm over C2
                mu = po.tile([128, 1], F32, tag="mu")
                nc.vector.reduce_sum(mu[:], o2[:], axis=mybir.AxisListType.X)
                nc.vector.tensor_scalar(mu[:], mu[:], 1.0 / C2, None, ALU.mult)
                xc = po.tile([128, C2], F32, tag="xc")
                nc.vector.tensor_scalar(xc[:], o2[:], mu[:], None,
                                        ALU.subtract)
                sq = po.tile([128, C2], F32, tag="sq")
                ssq = po.tile([128, 1], F32, tag="ssq")
                nc.scalar.activation(sq[:], xc[:], AF.Square, accum_out=ssq[:])
                sdev = po.tile([128, 1], F32, tag="sdev")
                nc.scalar.activation(sdev[:], ssq[:], AF.Sqrt,
                                     scale=1.0 / C2, bias=epsb[:, 0:1])
                rstd = po.tile([128, 1], F32, tag="rstd")
                nc.vector.reciprocal(rstd[:], sdev[:])
                xn = po.tile([128, C2], F32, tag="xn")
                nc.vector.tensor_scalar(xn[:], xc[:], rstd[:], None, ALU.mult)
                nc.vector.tensor_tensor(xn[:], xn[:], bc["gamma"][:],
                                        ALU.mult)
                ot = po.tile([128, C2], F32, tag="ot")
                nc.vector.tensor_tensor(ot[:], xn[:], bc["beta"][:], ALU.add)
                nc.sync.dma_start(out[w * WIN:(w + 1) * WIN, :], ot[:])

            if "1" in PH:
                edge_phase(1)
            if "2" in PH:
                edge_phase(2)

    nc.finalize()
    return nc


# --------------------------------------------------------------------------
_CACHE = {}


def run(inputs, trace=False):
    cfg, in_maps, slot_global = prep(**inputs)
    ckey = (cfg.key, os.environ.get("GAT_PHASES", "012"),
            os.environ.get("GAT_BUFS", "2"), os.environ.get("GAT_NWLIM"),
            os.environ.get("GAT_SCRATCH"), os.environ.get("GAT_GMAX"),
            os.environ.get("GAT_QUEUES"), os.environ.get("GAT_CHUNKS"))
    nc = _CACHE.get(ckey)
    if nc is None:
        nc = build(cfg)
        _CACHE[ckey] = nc
    if os.environ.get("GAT_SIM"):
        from concourse.bass_interp import MultiCoreSim
        nc.insert_bir_kernel_barrier_sem_inc()
        sim = MultiCoreSim(nc, CORES, aliases={}, require_finite=False,
                           require_nnan=False)
        innames = [a.memorylocations[0].name
                   for a in nc.m.functions[0].allocations
                   if getattr(a, 'kind', None) == 'ExternalInput']
        for t in range(CORES):
            for nm in innames:
                if nm == 'partition_id':
                    sim.cores[t].tensor(nm)[:] = np.array([[t]], np.uint32)
                else:
                    sim.cores[t].tensor(nm)[:] = in_maps[t][nm]
        sim.simulate()
        class R: pass
        res = R(); res.results = [
            {"out": np.array(sim.cores[t].tensor("out"))} for t in range(CORES)]
        res.exec_time_ns = None
    else:
        res = run_bass_kernel_spmd(nc, in_maps, core_ids=list(range(CORES)),
                                   trace=trace)
    full = np.concatenate([res.results[c]["out"] for c in range(CORES)],
                          axis=0)
    return full[slot_global], res


# --------------------------------------------------------------------------
# harness entry point
# --------------------------------------------------------------------------

def kernel(**inputs):
    """Full unsharded inputs -> full [N, 128] output (runs on 8 NeuronCores)."""
    out, _ = run(inputs)
    return out


# revision 52
# speedup vs baseline: 1.0498x; 1.0257x over previous
"""GATConv x2 + LayerNorm (GNN message passing) on 8 TRN2 NeuronCores.

Strategy (edge-parallel, dst-sharded):
  - Nodes are sharded across 8 cores by id range; each core owns all edges
    whose dst falls in its range (plus self loops).
  - Host assigns each core's nodes to 128-slot "windows", balancing total
    in-degree per window; edges are laid out per window in 128-edge blocks,
    split into low/high halves by source table row (int16 gather indices).
  - Phase 0 (per core): h1 = x @ W1 for own nodes, attention dot products
    e_src1/e_dst1; packed node table rows [h1 | e_src1] -> AllGather.
  - Phase 1: per window, dma_gather source rows (round-robin over 4 SWDGE
    queues so descriptor generation overlaps across Q7 core pairs), load
    host-precomputed one-hot matrices S (edge->dstslot) and S_T
    (dstslot->edge), broadcast per-window e_dst to edges via K tiny
    matmuls with S_T stationary, extract embedded e_src with the scalar
    engine, compute exp(leaky_relu(e_src+e_dst)) in bf16, one matmul per
    128-edge block with S stationary accumulates the weighted message sum
    and softmax denominators in PSUM.  Window epilogue normalizes, applies
    relu, computes h2 = relu(out1) @ W2 and the layer-2 attention dots;
    packed table 2 -> AllGather.
  - Phase 2: same edge machinery on table 2; epilogue does head-mean,
    bias, LayerNorm, and writes the output rows (window-slot order; host
    unpermutes).

Tables store h in bf16 with e-values embedded as f32 (bitcast slices).
"""

import sys

sys.path.insert(0, "/opt/trn_rl_repo")

import math
import os
import numpy as np
import ml_dtypes

import concourse.bass as bass
import concourse.bacc as bacc
import concourse.mybir as mybir
from concourse import tile
from concourse.bass_utils import run_bass_kernel_spmd

F32 = mybir.dt.float32
BF16 = mybir.dt.bfloat16
I16 = mybir.dt.int16
I32 = mybir.dt.int32
AF = mybir.ActivationFunctionType
ALU = mybir.AluOpType

CORES = 8
WIN = 128
H = 4

NEG_SLOPE = 0.2
EPS_LN = 1e-5


class Cfg:
    def __init__(self, N, IN_DIM, C1, C2, KL, KH, CH):
        assert N % CORES == 0
        self.N = N
        self.IN = IN_DIM          # input feature dim (=128)
        self.C1 = C1              # per-head dim layer 1 (64)
        self.C2 = C2              # per-head dim layer 2 (128)
        self.F1 = H * C1          # 256
        self.F2 = H * C2          # 512
        self.NODES_PC = N // CORES
        self.NW = math.ceil(self.NODES_PC / WIN)
        self.SLOTS_PC = self.NW * WIN
        self.TOT = CORES * self.SLOTS_PC
        assert self.TOT % 2 == 0
        self.HALF = self.TOT // 2
        assert self.HALF <= 32767, self.HALF
        assert self.SLOTS_PC <= 32767
        self.KL = KL
        self.KH = KH
        self.K = KL + KH
        # chunked AllGather: table rows are chunk-major
        # row(core,w,o) = crb[chunk(w)] + core*cw[chunk]*WIN + (w-w0)*WIN + o
        self.CH = CH
        # even chunks with a half-size tail: the last chunk's collective is
        # the only one exposed at the phase boundary, so keep it small
        last = max(1, self.NW // (2 * CH))
        rest = self.NW - last
        self.cw = [rest // (CH - 1) + (1 if i < rest % (CH - 1) else 0)
                   for i in range(CH - 1)] + [last] if CH > 1 else [self.NW]
        self.w0s = np.concatenate([[0], np.cumsum(self.cw)])
        self.crb = np.concatenate(
            [[0], np.cumsum([CORES * c * WIN for c in self.cw])])[:-1]
        # packed table rows (bf16 elements, 256B-multiple bytes)
        self.R1 = _pad_row(self.F1 * 2 + 16)   # bf16 cols
        self.R2 = _pad_row(self.F2 * 2 + 16)
        self.key = (N, IN_DIM, C1, C2, KL, KH, CH)

    def grow(self, core, w, o):
        """Global table row for (core, window, offset) — vectorized."""
        w = np.asarray(w)
        ch = np.searchsorted(self.w0s, w, side="right") - 1
        cw = np.asarray(self.cw)[ch]
        w0 = self.w0s[ch]
        return self.crb[ch] + core * cw * WIN + (w - w0) * WIN + o


def _pad_row(nbytes):
    """Round row bytes up to a multiple of 256; return bf16 col count."""
    b = ((nbytes + 255) // 256) * 256
    return b // 2


def _wrap_idx(arr):
    """[NWxL] int -> [128, NW*L/16] int16 wrapped+replicated per call."""
    nw, L = arr.shape
    assert L % 16 == 0
    w = arr.reshape(nw, L // 16, 16).transpose(0, 2, 1)  # [nw, 16, L/16]
    w = np.concatenate([w] * 8, axis=1)                  # [nw, 128, L/16]
    w = np.concatenate(list(w), axis=1)                  # [128, nw*L/16]
    return np.ascontiguousarray(w.astype(np.int16))


def prep(x, edge_index, W1, a_src1, a_dst1, b1, W2, a_src2, a_dst2, b2,
         gamma, beta):
    """Host-side sharding. Returns (cfg, in_maps, slot_global)."""
    N, IN_DIM = x.shape
    C1 = a_src1.shape[1]
    C2 = a_src2.shape[1]

    src = np.asarray(edge_index[0], dtype=np.int64)
    dst = np.asarray(edge_index[1], dtype=np.int64)
    loop = np.arange(N, dtype=np.int64)
    src = np.concatenate([src, loop])
    dst = np.concatenate([dst, loop])

    NODES_PC = N // CORES
    NW = math.ceil(NODES_PC / WIN)
    SLOTS_PC = NW * WIN

    # ---- window assignment per core (balance in-degree across NW bins) ----
    deg = np.bincount(dst, minlength=N)
    slot_global = np.empty(N, dtype=np.int64)
    win_of = np.empty(N, dtype=np.int64)    # window within core
    off_of = np.empty(N, dtype=np.int64)    # slot within window
    for c in range(CORES):
        nodes = np.arange(c * NODES_PC, (c + 1) * NODES_PC)
        d = deg[nodes]
        order = np.argsort(-d, kind="stable")
        # greedy: place next-heaviest node into least-loaded non-full bin
        bin_load = np.zeros(NW, dtype=np.int64)
        bin_cnt = np.zeros(NW, dtype=np.int64)
        wsel = np.empty(len(nodes), dtype=np.int64)
        osel = np.empty(len(nodes), dtype=np.int64)
        import heapq
        heap = [(0, 0, w) for w in range(NW)]
        heapq.heapify(heap)
        for i in order:
            while True:
                load, cnt, w = heapq.heappop(heap)
                if cnt < WIN:
                    break
            wsel[i] = w
            osel[i] = cnt
            heapq.heappush(heap, (load + d[i], cnt + 1, w))
        win_of[nodes] = wsel
        off_of[nodes] = osel
        slot_global[nodes] = c * SLOTS_PC + wsel * WIN + osel

    HALF = CORES * SLOTS_PC // 2
    CH = int(os.environ.get("GAT_CHUNKS", "4"))
    cfg0 = Cfg(N, IN_DIM, C1, C2, 0, 0, CH)

    owner = dst // NODES_PC
    src_core = src // NODES_PC

    # ---- pass 2: rebalance bins on (lo, hi) in-degree jointly ----
    # lo/hi labels from the pass-1 placement (approximation; final KL/KH
    # are recomputed from the final placement below)
    lo_lbl = (off_of[src] % 2) == 0
    dlo = np.bincount(dst[lo_lbl], minlength=N)
    dhi = np.bincount(dst[~lo_lbl], minlength=N)
    for c in range(CORES):
        nodes = np.arange(c * NODES_PC, (c + 1) * NODES_PC)
        dl, dh = dlo[nodes], dhi[nodes]
        order = np.argsort(-(dl + dh), kind="stable")
        binlo = np.zeros(NW, dtype=np.int64)
        binhi = np.zeros(NW, dtype=np.int64)
        bincnt = np.zeros(NW, dtype=np.int64)
        for i in order:
            cost = np.maximum(binlo + dl[i], binhi + dh[i]).astype(np.float64)
            cost[bincnt >= WIN] = np.inf
            w = int(np.argmin(cost))
            win_of[nodes[i]] = w
            off_of[nodes[i]] = bincnt[w]
            binlo[w] += dl[i]
            binhi[w] += dh[i]
            bincnt[w] += 1
        slot_global[nodes] = (c * SLOTS_PC + win_of[nodes] * WIN
                              + off_of[nodes])

    # ---- per-core edge layout (final placement) ----
    # Gathers are split by slot-offset PARITY (even/odd), which is identical
    # under both the core-major (t1) and chunk-major (t2) row numberings:
    # row = base*128 + o in both, so parity(row) = parity(o).  Each stream
    # gathers with elem_step = 2 rows, so int16 indices (row//2) cover the
    # whole table.  "lo" = even, "hi" = odd below.
    r1_of = slot_global                                  # core-major rows
    r2_of = cfg0.grow(np.arange(N) // NODES_PC, win_of, off_of)
    src_r1 = r1_of[src]
    src_r2 = r2_of[src]
    e_w = win_of[dst]
    e_off = off_of[dst]
    e_low = (src_r1 % 2) == 0

    # first pass: find KL / KH
    KL = 0
    KH = 0
    per_core = []
    for c in range(CORES):
        m = owner == c
        ew, eo, el = e_w[m], e_off[m], e_low[m]
        er1, er2 = src_r1[m] // 2, src_r2[m] // 2
        nlo = np.bincount(ew[el], minlength=NW)
        nhi = np.bincount(ew[~el], minlength=NW)
        KL = max(KL, int(np.ceil(nlo.max() / WIN)))
        KH = max(KH, int(np.ceil(nhi.max() / WIN)))
        per_core.append((ew, eo, er1, er2, el))
    cfg = Cfg(N, IN_DIM, C1, C2, KL, KH, CH)
    K = cfg.K

    in_maps = []
    for c in range(CORES):
        ew, eo, er1, er2, el = per_core[c]
        idx_lo1 = np.zeros((NW, KL * WIN), dtype=np.int64)
        idx_hi1 = np.zeros((NW, KH * WIN), dtype=np.int64)
        idx_lo2 = np.zeros((NW, KL * WIN), dtype=np.int64)
        idx_hi2 = np.zeros((NW, KH * WIN), dtype=np.int64)
        dstoff = np.full((NW, K * WIN), 999, dtype=np.int64)
        for w in range(NW):
            wm = ew == w
            lo_m = wm & el
            hi_m = wm & ~el
            # ascending source rows within each stream -> DMA reads walk
            # HBM mostly monotonically (better row-buffer locality)
            plo = np.argsort(er1[lo_m], kind="stable")
            phi = np.argsort(er1[hi_m], kind="stable")
            olo, ohi = eo[lo_m][plo], eo[hi_m][phi]
            idx_lo1[w, : len(olo)] = er1[lo_m][plo]
            idx_hi1[w, : len(ohi)] = er1[hi_m][phi]
            idx_lo2[w, : len(olo)] = er2[lo_m][plo]
            idx_hi2[w, : len(ohi)] = er2[hi_m][phi]
            dstoff[w, : len(olo)] = olo
            dstoff[w, KL * WIN : KL * WIN + len(ohi)] = ohi

        # one-hot matrices, [128, NW*K*128] bf16:
        #   Sg [e, (w,b,s)] = (dstoff[(w,b,e)] == s)   (edge-partition)
        #   SgT[s, (w,b,e)] = same                      (dstslot-partition)
        flat = dstoff.reshape(-1)                    # [(w,b,e)]
        j = np.arange(flat.size)
        valid = flat < WIN
        jv, fv = j[valid], flat[valid]
        L = NW * K * WIN
        Sg = np.zeros((WIN, L), dtype=ml_dtypes.bfloat16)
        Sg[jv % WIN, (jv // WIN) * WIN + fv] = 1
        SgT = np.zeros((WIN, L), dtype=ml_dtypes.bfloat16)
        SgT[fv, jv] = 1

        xs = np.zeros((SLOTS_PC, IN_DIM), dtype=np.float32)
        nodes = np.arange(c * NODES_PC, (c + 1) * NODES_PC)
        loc = win_of[nodes] * WIN + off_of[nodes]
        xs[loc] = np.asarray(x[nodes], dtype=np.float32)

        in_maps.append({
            "xT": np.ascontiguousarray(xs.T),
            "idx_lo1": _wrap_idx(idx_lo1),
            "idx_hi1": _wrap_idx(idx_hi1),
            "idx_lo2": _wrap_idx(idx_lo2),
            "idx_hi2": _wrap_idx(idx_hi2),
            "Sg": np.ascontiguousarray(Sg),
            "SgT": np.ascontiguousarray(SgT),
            "W1": np.asarray(W1, np.float32),
            "W2": np.asarray(W2, np.float32),
            "asrc1": np.asarray(a_src1, np.float32).reshape(1, -1),
            "adst1": np.asarray(a_dst1, np.float32).reshape(1, -1),
            "asrc2": np.asarray(a_src2, np.float32).reshape(1, -1),
            "adst2": np.asarray(a_dst2, np.float32).reshape(1, -1),
            "b1": np.asarray(b1, np.float32).reshape(1, -1),
            "b2": np.asarray(b2, np.float32).reshape(1, -1),
            "gamma": np.asarray(gamma, np.float32).reshape(1, -1),
            "beta": np.asarray(beta, np.float32).reshape(1, -1),
        })
    return cfg, in_maps, slot_global


# --------------------------------------------------------------------------
# device program
# --------------------------------------------------------------------------

def build(cfg):
    PH = os.environ.get("GAT_PHASES", "012")
    _sc = int(os.environ.get("GAT_SCRATCH", "16384"))
    _nq = int(os.environ.get("GAT_QUEUES", "4"))
    nc = bacc.Bacc("TRN2", target_bir_lowering=False, debug=False,
                   num_devices=CORES, dynamic_dma_scratch_size=_sc,
                   num_swdge_queues=_nq)
    NW, K, KL, KH = cfg.NW, cfg.K, cfg.KL, cfg.KH
    F1, F2, R1, R2 = cfg.F1, cfg.F2, cfg.R1, cfg.R2
    C1, C2 = cfg.C1, cfg.C2
    SL, TOT, HALF = cfg.SLOTS_PC, cfg.TOT, cfg.HALF
    RG = [list(range(CORES))]
    CH, cw, w0s, crb = cfg.CH, cfg.cw, cfg.w0s, cfg.crb
    chunk_end = {int(w0s[i + 1]) - 1: i for i in range(CH)}

    def emit_cc(shard, full, ch, ccc):
        w0, w1 = int(w0s[ch]), int(w0s[ch + 1])
        b = int(crb[ch])
        nc.gpsimd.collective_compute(
            "AllGather", ALU.bypass, replica_groups=RG,
            ins=[shard[w0 * WIN:w1 * WIN, :]],
            outs=[full[b:b + CORES * (w1 - w0) * WIN, :]])

    # ---- kernel I/O ----
    xT = nc.dram_tensor("xT", [cfg.IN, SL], F32, kind="ExternalInput")
    idx_lo1 = nc.dram_tensor("idx_lo1", [128, NW * KL * 8], I16, kind="ExternalInput")
    idx_hi1 = nc.dram_tensor("idx_hi1", [128, NW * KH * 8], I16, kind="ExternalInput")
    idx_lo2 = nc.dram_tensor("idx_lo2", [128, NW * KL * 8], I16, kind="ExternalInput")
    idx_hi2 = nc.dram_tensor("idx_hi2", [128, NW * KH * 8], I16, kind="ExternalInput")
    Sg = nc.dram_tensor("Sg", [128, NW * K * 128], BF16, kind="ExternalInput")
    SgT = nc.dram_tensor("SgT", [128, NW * K * 128], BF16, kind="ExternalInput")
    W1 = nc.dram_tensor("W1", [cfg.IN, F1], F32, kind="ExternalInput")
    W2 = nc.dram_tensor("W2", [F1, F2], F32, kind="ExternalInput")
    vecs = {}
    for nm, d in [("asrc1", F1), ("adst1", F1), ("asrc2", F2), ("adst2", F2),
                  ("b1", F1), ("b2", C2), ("gamma", C2), ("beta", C2)]:
        vecs[nm] = nc.dram_tensor(nm, [1, d], F32, kind="ExternalInput")
    out = nc.dram_tensor("out", [SL, C2], F32, kind="ExternalOutput")

    # ---- internal DRAM ----
    t1_shard = nc.dram_tensor("t1_shard", [SL, R1], BF16, kind="Internal")
    t2_shard = nc.dram_tensor("t2_shard", [SL, R2], BF16, kind="Internal")
    t1_full = nc.dram_tensor("t1_full", [TOT, R1], BF16, kind="Internal",
                             addr_space="Shared")
    t2_full = nc.dram_tensor("t2_full", [TOT, R2], BF16, kind="Internal",
                             addr_space="Shared")
    edst1 = nc.dram_tensor("edst1", [SL, H], BF16, kind="Internal")
    edst2 = nc.dram_tensor("edst2", [SL, H], BF16, kind="Internal")

    with tile.TileContext(nc) as tc:
        with tc.tile_pool(name="const", bufs=1) as cp:
            iota_i = cp.tile([128, 128], I32)
            nc.gpsimd.iota(iota_i[:], pattern=[[1, 128]], base=0,
                           channel_multiplier=0)
            iota_f = cp.tile([128, 128], F32)
            nc.vector.tensor_copy(iota_f[:], iota_i[:])
            ic_i = cp.tile([128, 1], I32)
            nc.gpsimd.iota(ic_i[:], pattern=[[0, 1]], base=0,
                           channel_multiplier=1)
            ic_f = cp.tile([128, 1], F32)
            nc.vector.tensor_copy(ic_f[:], ic_i[:])
            ident = cp.tile([128, 128], F32)
            nc.vector.tensor_scalar(ident[:], iota_f[:], ic_f[:, 0:1],
                                    None, ALU.is_equal)

            W1sb = cp.tile([128, F1], F32)
            nc.sync.dma_start(W1sb[:], W1[:, :])
            W2af = cp.tile([128, F2], F32)
            W2bf = cp.tile([128, F2], F32)
            nc.sync.dma_start(W2af[:], W2[0:128, :])
            nc.sync.dma_start(W2bf[:], W2[128:256, :])
            W2a = cp.tile([128, F2], BF16)
            W2b = cp.tile([128, F2], BF16)
            nc.vector.tensor_copy(W2a[:], W2af[:])
            nc.vector.tensor_copy(W2b[:], W2bf[:])
            identb = cp.tile([128, 128], BF16)
            nc.vector.tensor_copy(identb[:], ident[:])
            bcb = {}

            ones = cp.tile([1, 128], F32)
            nc.vector.memset(ones[:], 1.0)
            epsb = cp.tile([128, 1], F32)
            nc.vector.memset(epsb[:], EPS_LN)

            # broadcast small vectors to [128, D] via 1-row matmul
            bc = {}
            with tc.tile_pool(name="bcp", bufs=2, space="PSUM") as bps, \
                 tc.tile_pool(name="bcs", bufs=1) as bsb:
                for nm, d in [("asrc1", F1), ("adst1", F1), ("asrc2", F2),
                              ("adst2", F2), ("b1", F1), ("b2", C2),
                              ("gamma", C2), ("beta", C2)]:
                    vsb = bsb.tile([1, d], F32, tag="vload")
                    nc.sync.dma_start(vsb[:], vecs[nm][:, :])
                    t = cp.tile([128, d], F32, tag=f"bc_{nm}")
                    ps = bps.tile([128, d], F32, tag="bcps")
                    nc.tensor.matmul(ps[:], ones[:], vsb[:], start=True,
                                     stop=True)
                    nc.vector.tensor_copy(t[:], ps[:])
                    bc[nm] = t
                    if nm in ("asrc1", "adst1", "asrc2", "adst2"):
                        tb = cp.tile([128, d], BF16, tag=f"bcb_{nm}")
                        nc.vector.tensor_copy(tb[:], ps[:])
                        bcb[nm] = tb

            # ================= Phase 0: node tables =================
            with tc.tile_pool(name="p0", bufs=3) as p0, \
                 tc.tile_pool(name="p0ps", bufs=2, space="PSUM") as p0ps:
                for w in range(NW):
                    xt = p0.tile([128, 128], F32, tag="xt")
                    nc.sync.dma_start(xt[:], xT[:, w * WIN:(w + 1) * WIN])
                    h1 = p0ps.tile([128, F1], F32, tag="h1")
                    nc.tensor.matmul(h1[:], xt[:], W1sb[:], start=True,
                                     stop=True)
                    pk = p0.tile([128, R1], BF16, tag="pk")
                    nc.scalar.copy(pk[:, 0:F1], h1[:])
                    h1b = pk[:, 0:F1]
                    prod = p0.tile([128, F1], BF16, tag="prod")
                    es1 = p0.tile([128, H], F32, tag="es1")
                    nc.vector.tensor_tensor(prod[:], h1b, bcb["asrc1"][:],
                                            ALU.mult)
                    nc.vector.reduce_sum(
                        es1[:], prod[:].rearrange("p (h c) -> p h c", c=C1),
                        axis=mybir.AxisListType.X)
                    prod2 = p0.tile([128, F1], BF16, tag="prod2")
                    ed1f = p0.tile([128, H], F32, tag="ed1f")
                    ed1 = p0.tile([128, H], BF16, tag="ed1")
                    nc.vector.tensor_tensor(prod2[:], h1b, bcb["adst1"][:],
                                            ALU.mult)
                    nc.vector.reduce_sum(
                        ed1f[:], prod2[:].rearrange("p (h c) -> p h c", c=C1),
                        axis=mybir.AxisListType.X)
                    nc.vector.tensor_copy(ed1[:], ed1f[:])
                    nc.vector.tensor_copy(
                        pk[:, F1:F1 + 8].bitcast(F32), es1[:])
                    nc.sync.dma_start(
                        t1_shard[w * WIN:(w + 1) * WIN, 0:F1 + 8],
                        pk[:, 0:F1 + 8])
                    nc.sync.dma_start(edst1[w * WIN:(w + 1) * WIN, :],
                                      ed1[:])
            if "1" in PH or "2" in PH:
                # t1 is core-major: one fast AllGather (plain concat)
                nc.gpsimd.collective_compute(
                    "AllGather", ALU.bypass, replica_groups=RG,
                    ins=[t1_shard[:, :]], outs=[t1_full[:, :]])

            # ================= Phase 1 and 2 =================
            qn = [0]

            def edge_phase(layer):
                F = F1 if layer == 1 else F2
                C = C1 if layer == 1 else C2
                R = R1 if layer == 1 else R2
                tfull = t1_full if layer == 1 else t2_full
                edst = edst1 if layer == 1 else edst2
                sfx = f"L{layer}"
                _nb = int(os.environ.get("GAT_BUFS", "2"))
                _nwl = int(os.environ.get("GAT_NWLIM", str(NW)))
                GMAX = int(os.environ.get("GAT_GMAX", "4"))
                idx_lo = idx_lo1 if layer == 1 else idx_lo2
                idx_hi = idx_hi1 if layer == 1 else idx_hi2
                tf_ev = tfull[:, :].rearrange("(n two) r -> n (two r)",
                                              two=2)[:, 0:R]
                tf_od = tfull[:, :].rearrange("(n two) r -> n (two r)",
                                              two=2)[:, R:2 * R]
                with tc.tile_pool(name=f"pe{sfx}", bufs=_nb) as pe, \
                     tc.tile_pool(name=f"peps{sfx}", bufs=2, space="PSUM") as pps, \
                     tc.tile_pool(name=f"po{sfx}", bufs=_nb) as po, \
                     tc.tile_pool(name=f"pops{sfx}", bufs=2, space="PSUM") as ops:
                    for w in range(min(NW, _nwl)):
                        ilo = pe.tile([128, KL * 8], I16, tag="ilo")
                        nc.sync.dma_start(
                            ilo[:], idx_lo[:, w * KL * 8:(w + 1) * KL * 8])
                        ihi = pe.tile([128, KH * 8], I16, tag="ihi")
                        nc.sync.dma_start(
                            ihi[:], idx_hi[:, w * KH * 8:(w + 1) * KH * 8])
                        Ssb = pe.tile([128, K * 128], BF16, tag="Ssb")
                        nc.sync.dma_start(
                            Ssb[:], Sg[:, w * K * 128:(w + 1) * K * 128])
                        STsb = pe.tile([128, K * 128], BF16, tag="STsb")
                        nc.sync.dma_start(
                            STsb[:], SgT[:, w * K * 128:(w + 1) * K * 128])
                        edw = pe.tile([128, H], BF16, tag="edw")
                        nc.sync.dma_start(edw[:],
                                          edst[w * WIN:(w + 1) * WIN, :])

                        glo = pe.tile([128, KL * R], BF16, tag="glo")
                        ghi = pe.tile([128, KH * R], BF16, tag="ghi")

                        def gcalls(tile_, src_ap, idx_tile, nblk, elem):
                            gv = tile_[:].rearrange("p (t e) -> p t e",
                                                    e=elem)
                            for g0 in range(0, nblk, GMAX):
                                nb = min(GMAX, nblk - g0)
                                nc.gpsimd.dma_gather(
                                    gv[:, g0:g0 + nb, :], src_ap,
                                    idx_tile[:, g0 * 8:(g0 + nb) * 8],
                                    nb * WIN, nb * WIN, elem,
                                    elem_step=2 * elem,
                                    queue_num=qn[0] % _nq)
                                qn[0] += 1

                        gcalls(glo, tf_ev, ilo, KL, R)
                        gcalls(ghi, tf_od, ihi, KH, R)

                        # per-edge e_dst via K tiny matmuls (S_T stationary)
                        psE = pps.tile([128, K * H], F32, tag="psE")
                        for b in range(K):
                            nc.tensor.matmul(
                                psE[:, b * H:(b + 1) * H],
                                STsb[:, b * 128:(b + 1) * 128], edw[:],
                                start=True, stop=True)

                        # embedded e_src -> compact (split vector/scalar)
                        es_c = pe.tile([128, K * H], F32, tag="es_c")
                        esv = es_c[:].rearrange("p (k h) -> p k h", h=H)
                        nc.vector.tensor_copy(
                            esv[:, 0:KL, :],
                            glo[:].rearrange("p (t e) -> p t e", e=R)
                                [:, :, F:F + 8].bitcast(F32))
                        nc.scalar.copy(
                            esv[:, KL:K, :],
                            ghi[:].rearrange("p (t e) -> p t e", e=R)
                                [:, :, F:F + 8].bitcast(F32))

                        e_all = pe.tile([128, K * H], F32, tag="e_all")
                        nc.vector.tensor_tensor(e_all[:], es_c[:], psE[:],
                                                ALU.add)
                        e_sc = pe.tile([128, K * H], F32, tag="e_sc")
                        nc.vector.tensor_scalar(e_sc[:], e_all[:], NEG_SLOPE,
                                                None, ALU.mult)
                        nc.vector.tensor_tensor(e_all[:], e_all[:], e_sc[:],
                                                ALU.max)
                        w_all = pe.tile([128, K * H], BF16, tag="w_all")
                        nc.scalar.activation(w_all[:], e_all[:], AF.Exp)
                        # exp with broadcast input -> pre-expanded weights so
                        # the big multiply below is contiguous bf16 (2x DVE)
                        w_exp = pe.tile([128, K * F], BF16, tag="w_exp")
                        nc.scalar.activation(
                            w_exp[:].rearrange("p (k h c) -> p k h c", h=H,
                                               c=C),
                            e_all[:].rearrange("p (k h) -> p k h", h=H)
                                .unsqueeze(-1).broadcast_to([128, K, H, C]),
                            AF.Exp)

                        RC = F + H  # rhs cols per block
                        rhs = pe.tile([128, K * RC], BF16, tag="rhs")
                        rv = rhs[:].rearrange("p (k r) -> p k r", r=RC)
                        wv = w_all[:].rearrange("p (k h) -> p k h", h=H)
                        wev = w_exp[:].rearrange("p (k f) -> p k f", f=F)
                        nc.vector.tensor_tensor(
                            rv[:, 0:KL, 0:F],
                            glo[:].rearrange("p (t e) -> p t e", e=R)
                                [:, :, 0:F],
                            wev[:, 0:KL, :], ALU.mult)
                        nc.vector.tensor_tensor(
                            rv[:, KL:K, 0:F],
                            ghi[:].rearrange("p (t e) -> p t e", e=R)
                                [:, :, 0:F],
                            wev[:, KL:K, :], ALU.mult)
                        nc.vector.tensor_copy(rv[:, :, F:F + H], wv)

                        if layer == 1:
                            psW = pps.tile([128, F + H], F32, tag="psW")
                            for b in range(K):
                                nc.tensor.matmul(
                                    psW[:], Ssb[:, b * 128:(b + 1) * 128],
                                    rv[:, b, :], start=(b == 0),
                                    stop=(b == K - 1))
                            _epilogue1(nc, tc, po, ops, psW[:, 0:F1],
                                       psW[:, F1:F1 + H], bc, W2a,
                                       W2b, ident, cfg, w, t2_shard, edst2)
                            if "2" in PH and w in chunk_end:
                                emit_cc(t2_shard, t2_full, chunk_end[w], F2 + 8)
                        else:
                            psA = pps.tile([128, F2], F32, tag="psA")
                            psD = pps.tile([128, H], F32, tag="psD")
                            for b in range(K):
                                S_b = Ssb[:, b * 128:(b + 1) * 128]
                                nc.tensor.matmul(
                                    psA[:], S_b, rv[:, b, 0:F],
                                    start=(b == 0), stop=(b == K - 1))
                                nc.tensor.matmul(
                                    psD[:], S_b, rv[:, b, F:F + H],
                                    start=(b == 0), stop=(b == K - 1))
                            _epilogue2(nc, po, psA, psD, bc, cfg, w, out)

            def _epilogue1(nc, tc, po, ops, psA, psD, bc, W2a, W2b, ident,
                           cfg, w, t2_shard, edst2):
                den = po.tile([128, H], F32, tag="den")
                nc.vector.tensor_scalar(den[:], psD[:], 1e-30,
                                        None, ALU.max)
                rec = po.tile([128, H], F32, tag="rec")
                nc.vector.reciprocal(rec[:], den[:])
                o1 = po.tile([128, F1], F32, tag="o1")
                nc.vector.tensor_tensor(
                    o1[:].rearrange("p (h c) -> p h c", c=C1),
                    psA[:].rearrange("p (h c) -> p h c", c=C1),
                    rec[:].unsqueeze(-1).broadcast_to([128, H, C1]),
                    ALU.mult)
                nc.vector.tensor_tensor(o1[:], o1[:], bc["b1"][:], ALU.add)
                o1b = po.tile([128, F1], BF16, tag="o1b")
                nc.scalar.activation(o1b[:], o1[:], AF.Relu)
                tp = ops.tile([128, 128], BF16, tag="tp")
                t0 = po.tile([128, 128], BF16, tag="t0")
                nc.tensor.transpose(tp[:], o1b[:, 0:128], identb[:])
                nc.vector.tensor_copy(t0[:], tp[:])
                tp2 = ops.tile([128, 128], BF16, tag="tp")
                t1t = po.tile([128, 128], BF16, tag="t1t")
                nc.tensor.transpose(tp2[:], o1b[:, 128:256], identb[:])
                nc.vector.tensor_copy(t1t[:], tp2[:])
                h2 = ops.tile([128, F2], F32, tag="h2")
                nc.tensor.matmul(h2[:], t0[:], W2a[:], start=True, stop=False)
                nc.tensor.matmul(h2[:], t1t[:], W2b[:], start=False, stop=True)
                pk2 = po.tile([128, R2], BF16, tag="pk2")
                nc.scalar.copy(pk2[:, 0:F2], h2[:])
                h2b = pk2[:, 0:F2]
                pr = po.tile([128, F2], BF16, tag="pr")
                es2 = po.tile([128, H], F32, tag="es2")
                nc.vector.tensor_tensor(pr[:], h2b, bcb["asrc2"][:], ALU.mult)
                nc.vector.reduce_sum(
                    es2[:], pr[:].rearrange("p (h c) -> p h c", c=C2),
                    axis=mybir.AxisListType.X)
                pr2 = po.tile([128, F2], BF16, tag="pr2")
                ed2f = po.tile([128, H], F32, tag="ed2f")
                ed2 = po.tile([128, H], BF16, tag="ed2")
                nc.vector.tensor_tensor(pr2[:], h2b, bcb["adst2"][:],
                                        ALU.mult)
                nc.vector.reduce_sum(
                    ed2f[:], pr2[:].rearrange("p (h c) -> p h c", c=C2),
                    axis=mybir.AxisListType.X)
                nc.vector.tensor_copy(ed2[:], ed2f[:])
                nc.vector.tensor_copy(pk2[:, F2:F2 + 8].bitcast(F32), es2[:])
                nc.sync.dma_start(t2_shard[w * WIN:(w + 1) * WIN, 0:F2 + 8],
                                  pk2[:, 0:F2 + 8])
                nc.sync.dma_start(edst2[w * WIN:(w + 1) * WIN, :], ed2[:])

            def _epilogue2(nc, po, psA, psD, bc, cfg, w, out):
                den = po.tile([128, H], F32, tag="den2")
                nc.vector.tensor_scalar(den[:], psD[:], 1e-30, None, ALU.max)
                rec = po.tile([128, H], F32, tag="rec2")
                nc.vector.reciprocal(rec[:], den[:])
                tmp = po.tile([128, F2], F32, tag="tmp")
                nc.vector.tensor_tensor(
                    tmp[:].rearrange("p (h c) -> p h c", c=C2),
                    psA[:].rearrange("p (h c) -> p h c", c=C2),
                    rec[:].unsqueeze(-1).broadcast_to([128, H, C2]),
                    ALU.mult)
                s01 = po.tile([128, C2], F32, tag="s01")
                nc.vector.tensor_tensor(s01[:], tmp[:, 0:C2],
                                        tmp[:, C2:2 * C2], ALU.add)
                s23 = po.tile([128, C2], F32, tag="s23")
                nc.vector.tensor_tensor(s23[:], tmp[:, 2 * C2:3 * C2],
                                        tmp[:, 3 * C2:4 * C2], ALU.add)
                o2 = po.tile([128, C2], F32, tag="o2")
                nc.vector.tensor_tensor(o2[:], s01[:], s23[:], ALU.add)
                nc.vector.tensor_scalar(o2[:], o2[:], 1.0 / H, None, ALU.mult)
                nc.vector.tensor_tensor(o2[:], o2[:], bc["b2"][:], ALU.add)
                # LayerNorm over C2
                mu = po.tile([128, 1], F32, tag="mu")
                nc.vector.reduce_sum(mu[:], o2[:], axis=mybir.AxisListType.X)
                nc.vector.tensor_scalar(mu[:], mu[:], 1.0 / C2, None, ALU.mult)
                xc = po.tile([128, C2], F32, tag="xc")
                nc.vector.tensor_scalar(xc[:], o2[:], mu[:], None,
                                        ALU.subtract)
                sq = po.tile([128, C2], F32, tag="sq")
                ssq = po.tile([128, 1], F32, tag="ssq")
                nc.scalar.activation(sq[:], xc[:], AF.Square, accum_out=ssq[:])
                sdev = po.tile([128, 1], F32, tag="sdev")
                nc.scalar.activation(sdev[:], ssq[:], AF.Sqrt,
                                     scale=1.0 / C2, bias=epsb[:, 0:1])
                rstd = po.tile([128, 1], F32, tag="rstd")
                nc.vector.reciprocal(rstd[:], sdev[:])
                xn = po.tile([128, C2], F32, tag="xn")
                nc.vector.tensor_scalar(xn[:], xc[:], rstd[:], None, ALU.mult)
                nc.vector.tensor_tensor(xn[:], xn[:], bc["gamma"][:],
                                        ALU.mult)
                ot = po.tile([128, C2], F32, tag="ot")
                nc.vector.tensor_tensor(ot[:], xn[:], bc["beta"][:], ALU.add)
                nc.sync.dma_start(out[w * WIN:(w + 1) * WIN, :], ot[:])

            if "1" in PH:
                edge_phase(1)
            if "2" in PH:
                edge_phase(2)

    nc.finalize()
    return nc


# --------------------------------------------------------------------------
_CACHE = {}


def run(inputs, trace=False):
    cfg, in_maps, slot_global = prep(**inputs)
    ckey = (cfg.key, os.environ.get("GAT_PHASES", "012"),
            os.environ.get("GAT_BUFS", "2"), os.environ.get("GAT_NWLIM"),
            os.environ.get("GAT_SCRATCH"), os.environ.get("GAT_GMAX"),
            os.environ.get("GAT_QUEUES"), os.environ.get("GAT_CHUNKS"))
    nc = _CACHE.get(ckey)
    if nc is None:
        nc = build(cfg)
        _CACHE[ckey] = nc
    if os.environ.get("GAT_SIM"):
        from concourse.bass_interp import MultiCoreSim
        nc.insert_bir_kernel_barrier_sem_inc()
        sim = MultiCoreSim(nc, CORES, aliases={}, require_finite=False,
                           require_nnan=False)
        innames = [a.memorylocations[0].name
                   for a in nc.m.functions[0].allocations
                   if getattr(a, 'kind', None) == 'ExternalInput']
        for t in range(CORES):
            for nm in innames:
                if nm == 'partition_id':
                    sim.cores[t].tensor(nm)[:] = np.array([[t]], np.uint32)
                else:
                    sim.cores[t].tensor(nm)[:] = in_maps[t][nm]
        sim.simulate()
        class R: pass
        res = R(); res.results = [
            {"out": np.array(sim.cores[t].tensor("out"))} for t in range(CORES)]
        res.exec_time_ns = None
    else:
        res = run_bass_kernel_spmd(nc, in_maps, core_ids=list(range(CORES)),
                                   trace=trace)
    full = np.concatenate([res.results[c]["out"] for c in range(CORES)],
                          axis=0)
    return full[slot_global], res


# --------------------------------------------------------------------------
# harness entry point
# --------------------------------------------------------------------------

def kernel(**inputs):
    """Full unsharded inputs -> full [N, 128] output (runs on 8 NeuronCores)."""
    out, _ = run(inputs)
    return out
